# revision 1
# baseline (speedup 1.0000x reference)
"""Trainium2 Bass kernel for MoE-LoRA GQA attention (nn_Attention_57389353009692).

Strategy (8 NeuronCores, one SPMD launch):
  - Tensor-parallel over heads: core c owns q-heads 4c..4c+3 and kv-head c
    (GQA repeat_interleave aligns head h with kv-head h//4).
  - Each core computes its QKV projections (+ MoE-LoRA), RoPE, and flash-style
    attention for its heads over the full sequence, producing the attention
    output in feature-major layout [256 feat, 2048 tok] (bf16).
  - One AllToAll reshards from head-sharded to sequence-sharded: core c ends
    up with out[:, 256c:256(c+1)] == full feature dim for its 256 tokens.
  - Each core then does the output projection + o-LoRA for its 256 tokens.
  - Host concatenates the 8 row-blocks.

Numerics: fp32 DMA'd inputs are cast to bf16 on host for matmul operands;
accumulation is fp32 in PSUM; softmax (attention + router) runs in fp32.
Attention softmax uses exp() without max-subtraction — scores are O(1) for
this problem's input distribution (0.02-scaled weights); the mask is clamped
to -1e30 on host so exp() underflows to exactly 0 for masked entries.
Scale 1/sqrt(64) is folded into wq (and the q-LoRA B) on host.

RoPE trick: the interleaved even/odd pairing is turned into contiguous
half-blocks by permuting wq/wk output features on host (per 2-head "page":
[h0 evens | h1 evens | h0 odds | h1 odds]), so RoPE is plain full-width
vector ops; a small SBUF rearrange then makes each head's 64 dims contiguous
for the score matmuls.
"""

import sys

for _p in ("/opt/trn_rl_repo", "/root/.axon_site/_ro/trn_rl_repo"):
    if _p not in sys.path:
        sys.path.insert(0, _p)

import numpy as np
import ml_dtypes

import concourse.bass as bass
import concourse.tile as tile
from concourse import bacc, mybir
from concourse.masks import make_identity
from concourse.alu_op_type import AluOpType

F32 = mybir.dt.float32
BF16 = mybir.dt.bfloat16
AF = mybir.ActivationFunctionType
AX = mybir.AxisListType
BF16NP = ml_dtypes.bfloat16

B, S, D = 1, 2048, 2048
H, KVH, HD = 32, 8, 64
NREP = H // KVH
R, E = 8, 8
SCALING = 32.0 / 8.0
NCORES = 8
QH = H // NCORES          # 4 q heads per core
QF = QH * HD              # 256 q feats per core
KF = HD                   # 64 kv feats per core
TSH = S // NCORES         # 256 tokens per core for o-proj
NKT = S // 128            # 16 key tiles
NQB = S // 512            # 4 query blocks
NIF = D // 128            # 16 contraction tiles

MASK_NEG = -1e30

# mask tile classes
M_SKIP, M_ZERO, M_ADD = 0, 1, 2
BUILD_MODE = "ALL"  # debug: A | C | CC | ALL


def _build_perm():
    """Per-core feature permutations for rope-friendly layout."""
    idx_q = np.zeros(QF, dtype=np.int64)
    for f in range(QF):
        page, w = divmod(f, 128)
        if w < 32:
            hl, j, odd = 2 * page, w, 0
        elif w < 64:
            hl, j, odd = 2 * page + 1, w - 32, 0
        elif w < 96:
            hl, j, odd = 2 * page, w - 64, 1
        else:
            hl, j, odd = 2 * page + 1, w - 96, 1
        idx_q[f] = 64 * hl + 2 * j + odd
    idx_k = np.zeros(KF, dtype=np.int64)
    for w in range(KF):
        if w < 32:
            idx_k[w] = 2 * w
        else:
            idx_k[w] = 2 * (w - 32) + 1
    return idx_q, idx_k


IDX_Q, IDX_K = _build_perm()

# head h (local 0..3) lives at page h//2, partitions (h%2)*64 .. +64 after the
# head-contiguous rearrange.


def _lora_pack(A, router):
    """[E,R,D] A + [E,D] router -> [D, 72] stationary (cols r*8+e | 64+e)."""
    ap = np.transpose(A, (1, 0, 2)).reshape(E * R, -1).T  # [D, 64]
    return np.concatenate([ap, router.T], axis=1)  # [D, 72]


def _b_flat(Bw, scale):
    """[E, OF, R] -> [64, OF] with row r*8+e."""
    return (np.transpose(Bw, (2, 0, 1)).reshape(E * R, -1) * scale)


def _bf(x):
    return np.ascontiguousarray(x, dtype=np.float32).astype(BF16NP)


def _f32(x):
    return np.ascontiguousarray(x, dtype=np.float32)


def classify_mask(maskT):
    """maskT: [S(k), S(q)] clamped fp32. Returns [NKT, NQB] class map."""
    cls = np.zeros((NKT, NQB), dtype=np.int64)
    for kt in range(NKT):
        blk_rows = maskT[kt * 128:(kt + 1) * 128]
        for qb in range(NQB):
            blk = blk_rows[:, qb * 512:(qb + 1) * 512]
            if np.all(blk <= MASK_NEG * 0.5):
                cls[kt, qb] = M_SKIP
            elif np.all(blk == 0.0):
                cls[kt, qb] = M_ZERO
            else:
                cls[kt, qb] = M_ADD
    return cls


def build(mask_cls):
    """Build the SPMD Bass graph. mask_cls: [NKT, NQB] int array."""
    nc = bacc.Bacc(None, target_bir_lowering=False)

    # ---- DRAM I/O (per-core shards prepared on host) ----
    xT = nc.declare_dram_parameter("xT", [D, S], BF16, isOutput=False)
    wqT = nc.declare_dram_parameter("wqT", [D, QF], BF16, isOutput=False)
    wkT = nc.declare_dram_parameter("wkT", [D, KF], BF16, isOutput=False)
    wvT = nc.declare_dram_parameter("wvT", [D, KF], BF16, isOutput=False)
    aq = nc.declare_dram_parameter("aq", [D, 72], BF16, isOutput=False)
    ak = nc.declare_dram_parameter("ak", [D, 72], BF16, isOutput=False)
    av = nc.declare_dram_parameter("av", [D, 72], BF16, isOutput=False)
    ao = nc.declare_dram_parameter("ao", [D, 72], BF16, isOutput=False)
    bq = nc.declare_dram_parameter("bq", [E * R, QF], BF16, isOutput=False)
    bk = nc.declare_dram_parameter("bk", [E * R, KF], BF16, isOutput=False)
    bv = nc.declare_dram_parameter("bv", [E * R, KF], BF16, isOutput=False)
    bo = nc.declare_dram_parameter("bo", [E * R, D], BF16, isOutput=False)
    woT = nc.declare_dram_parameter("woT", [D, D], BF16, isOutput=False)
    cs2 = nc.declare_dram_parameter("cs2", [64, S], BF16, isOutput=False)
    sn2 = nc.declare_dram_parameter("sn2", [64, S], BF16, isOutput=False)
    maskT = nc.declare_dram_parameter("maskT", [S, S], BF16, isOutput=False)
    sel = nc.declare_dram_parameter("sel", [H, NIF * 128], F32,
                                    isOutput=False)
    y = nc.declare_dram_parameter("y", [TSH, D], F32, isOutput=True)

    # internal DRAM for the collective
    cc_in = nc.dram_tensor("cc_in", [NCORES, QF + QH, TSH], BF16)
    cc_out = nc.dram_tensor("cc_out", [NCORES, QF + QH, TSH], BF16)

    with tile.TileContext(nc) as tc:
        _emit(nc, tc, locals(), mask_cls)
    nc.finalize()
    return nc


def _emit(nc, tc, t, mask_cls):
    xT, wqT, wkT, wvT = t["xT"], t["wqT"], t["wkT"], t["wvT"]
    aq, ak, av, ao = t["aq"], t["ak"], t["av"], t["ao"]
    bq, bk, bv, bo = t["bq"], t["bk"], t["bv"], t["bo"]
    woT, cs2, sn2, maskT, y = t["woT"], t["cs2"], t["sn2"], t["maskT"], t["y"]
    sel = t["sel"]
    cc_in, cc_out = t["cc_in"], t["cc_out"]

    import contextlib
    ctx = contextlib.ExitStack()
    with ctx:
        persist = ctx.enter_context(tc.tile_pool(name="persist", bufs=1))
        ps = ctx.enter_context(tc.tile_pool(name="ps", bufs=1, space="PSUM"))

        # ---- persistent tiles (weights + attention operands) ----
        wqT_sb = persist.tile([128, NIF, QF], BF16)
        nc.sync.dma_start(out=wqT_sb, in_=wqT.rearrange("(n p) f -> p n f", p=128))
        a_sb = {}
        a_sb["q"] = persist.tile([128, NIF, 72], BF16, name="a_q", tag="a_q")
        nc.sync.dma_start(out=a_sb["q"],
                          in_=aq.rearrange("(n p) f -> p n f", p=128))
        wkT_sb = persist.tile([128, NIF, KF], BF16)
        nc.scalar.dma_start(out=wkT_sb,
                            in_=wkT.rearrange("(n p) f -> p n f", p=128))
        wvT_sb = persist.tile([128, NIF, KF], BF16)
        nc.scalar.dma_start(out=wvT_sb,
                            in_=wvT.rearrange("(n p) f -> p n f", p=128))
        for name, hnd in (("k", ak), ("v", av), ("o", ao)):
            a_sb[name] = persist.tile([128, NIF, 72], BF16,
                                      name="a_" + name, tag="a_" + name)
            nc.gpsimd.dma_start(out=a_sb[name],
                                in_=hnd.rearrange("(n p) f -> p n f", p=128))
        bq_sb = persist.tile([64, QF], BF16)
        nc.gpsimd.dma_start(out=bq_sb, in_=bq[:])
        bk_sb = persist.tile([64, KF], BF16)
        nc.gpsimd.dma_start(out=bk_sb, in_=bk[:])
        bv_sb = persist.tile([64, KF], BF16)
        nc.gpsimd.dma_start(out=bv_sb, in_=bv[:])
        bo_sb = persist.tile([64, D], BF16)
        nc.gpsimd.dma_start(out=bo_sb, in_=bo[:])
        cs_sb = persist.tile([64, S], BF16)
        nc.gpsimd.dma_start(out=cs_sb, in_=cs2[:])
        sn_sb = persist.tile([64, S], BF16)
        nc.gpsimd.dma_start(out=sn_sb, in_=sn2[:])
        sel_sb = persist.tile([H, NIF * 128], F32)
        nc.gpsimd.dma_start(out=sel_sb, in_=sel[:])

        ident_f = persist.tile([128, 128], F32)
        make_identity(nc, ident_f)
        ident_b = persist.tile([128, 128], BF16)
        make_identity(nc, ident_b)

        # head-contiguous rotated q/k; heads at partition base 64*(h%2),
        # page h//2 — enables 2-head row-packed score matmuls. kh is
        # duplicated into both partition halves (GQA: one kv head serves
        # all four q heads).
        qh_sb = persist.tile([128, 2, S], BF16)
        kh_sb = persist.tile([128, S], BF16)
        vT_sb = persist.tile([64, S], BF16)
        vtok = persist.tile([128, NKT, 65], BF16)  # token-major v + ones col
        g_sb = persist.tile([128, NIF, TSH], BF16)  # gathered out (post-A2A)

        def lora_rw(pool, dpool, h_ps, ntok, tag):
            """Router softmax from logits rows [64:72) of h_ps ([72, ntok]).

            Returns sbuf [64, ntok] f32 with row r*8+e = rw[:, e], scaled x1.
            """
            nch = ntok // 128
            lgT = pool.tile([8, ntok], F32, name="lgT", tag="lgT", bufs=2)
            nc.vector.tensor_copy(lgT, h_ps[64:72, :])
            lgtok_ps = ps.tile([128, 8 * nch], F32, name="lgtok_ps", tag="b_tp")
            for chk in range(nch):
                nc.tensor.transpose(
                    lgtok_ps[:, 8 * chk:8 * chk + 8],
                    lgT[:, 128 * chk:128 * chk + 128],
                    ident_f[0:8, 0:8],
                )
            lgtok = pool.tile([128, nch, 8], F32, name="lgtok", tag="lgtok", bufs=2)
            nc.vector.tensor_copy(lgtok, lgtok_ps.rearrange("p (n e) -> p n e", e=8))
            mx = pool.tile([128, nch], F32, name="mx", tag="mx", bufs=2)
            nc.vector.tensor_reduce(mx, lgtok, axis=AX.X, op=AluOpType.max)
            lgs = pool.tile([128, nch, 8], F32, name="lgs", tag="lgs", bufs=2)
            nc.vector.tensor_tensor(lgs, lgtok,
                                    mx.unsqueeze(2).broadcast_to([128, nch, 8]),
                                    AluOpType.subtract)
            ex = pool.tile([128, nch, 8], F32, name="ex", tag="ex", bufs=2)
            nc.scalar.activation(ex, lgs, AF.Exp)
            sm = pool.tile([128, nch], F32, name="sm", tag="sm", bufs=2)
            nc.vector.tensor_reduce(sm, ex, axis=AX.X, op=AluOpType.add)
            rc = pool.tile([128, nch], F32, name="rc", tag="rc", bufs=2)
            nc.vector.reciprocal(rc, sm)
            rw = pool.tile([128, nch, 8], F32, name="rw", tag="rw", bufs=2)
            nc.vector.tensor_tensor(rw, ex,
                                    rc.unsqueeze(2).broadcast_to([128, nch, 8]),
                                    AluOpType.mult)
            rwT_ps = ps.tile([8, ntok], F32, name="rwT_ps", tag="b_tp")
            for chk in range(nch):
                nc.tensor.transpose(
                    rwT_ps[:, 128 * chk:128 * chk + 128],
                    rw[:, chk, :],
                    ident_f[:, 0:128],
                )
            rwT = pool.tile([8, ntok], F32, name="rwT", tag="rwT", bufs=2)
            nc.vector.tensor_copy(rwT, rwT_ps)
            rw_dr = dpool.tile([8, ntok], F32, name="rw_dr", tag="rw_dr",
                               bufs=2)
            nc.scalar.dma_start(out=rw_dr, in_=rwT)
            rwx = pool.tile([64, ntok], F32, name="rwx", tag="rwx", bufs=2)
            nc.scalar.dma_start(
                out=rwx,
                in_=bass.AP(tensor=rw_dr.tensor, offset=rw_dr.offset,
                            ap=[[0, R], [ntok, R], [1, ntok]]))
            return rwx

        # ================= Phase A+B: QKV + LoRA + RoPE =================
        with tc.tile_pool(name="pA", bufs=1) as pA, \
                tc.tile_pool(name="pAd", bufs=2, space="DRAM") as pAd:
            # layout: [64 part, half(e/o), page, S] — keeps tensor ops at
            # base partition 0 (walrus: tensor_tensor operands must share
            # start partition)
            q_pre = pA.tile([64, 2, 2, S], F32)
            k_pre = pA.tile([32, 2, S], F32)
            qrot = pA.tile([64, 2, 2, S], BF16)
            krot = pA.tile([32, 2, S], BF16)

            for tb in range(4):
                tsl = slice(tb * 512, (tb + 1) * 512)
                xq = pA.tile([128, NIF, 512], BF16, name="xq", tag="xq",
                             bufs=2)
                nc.scalar.dma_start(
                    out=xq,
                    in_=xT.rearrange("(n p) t -> p n t", p=128)[:, :, tsl])
                # --- sub-phase A1: q projection + q-LoRA ---
                q0 = ps.tile([128, 512], F32, name="q0", tag="b_q0")
                q1 = ps.tile([128, 512], F32, name="q1", tag="b_q1")
                hq = ps.tile([72, 512], F32, name="hq", tag="b_hq")
                for k in range(NIF):
                    nc.tensor.matmul(hq, a_sb["q"][:, k, :], xq[:, k, :],
                                     start=(k == 0), stop=(k == NIF - 1))
                for k in range(NIF):
                    rhs = xq[:, k, :]
                    st = k == 0
                    nc.tensor.matmul(q0, wqT_sb[:, k, 0:128], rhs,
                                     start=st, stop=False)
                    nc.tensor.matmul(q1, wqT_sb[:, k, 128:256], rhs,
                                     start=st, stop=False)
                rwxq = lora_rw(pA, pAd, hq, 512, "q")
                hpq = pA.tile([64, 512], BF16, name="hpq", tag="hp", bufs=2)
                nc.vector.tensor_tensor(hpq, hq[0:64, :], rwxq, AluOpType.mult)
                nc.tensor.matmul(q0, bq_sb[:, 0:128], hpq, start=False, stop=True)
                nc.tensor.matmul(q1, bq_sb[:, 128:256], hpq, start=False,
                                 stop=True)
                nc.vector.tensor_copy(q_pre[:, 0, 0, tsl], q0[0:64, :])
                nc.vector.tensor_copy(q_pre[:, 1, 0, tsl], q0[64:128, :])
                nc.vector.tensor_copy(q_pre[:, 0, 1, tsl], q1[0:64, :])
                nc.vector.tensor_copy(q_pre[:, 1, 1, tsl], q1[64:128, :])

                # --- sub-phase A2: k/v projections + their LoRAs ---
                kp = ps.tile([64, 512], F32, name="kp", tag="b_kp")
                vp = ps.tile([64, 512], F32, name="vp", tag="b_vp")
                hk = ps.tile([72, 512], F32, name="hk", tag="b_hk")
                hv = ps.tile([72, 512], F32, name="hv", tag="b_hv")
                for k in range(NIF):
                    st = k == 0
                    sp = k == NIF - 1
                    nc.tensor.matmul(hk, a_sb["k"][:, k, :], xq[:, k, :],
                                     start=st, stop=sp)
                    nc.tensor.matmul(hv, a_sb["v"][:, k, :], xq[:, k, :],
                                     start=st, stop=sp)
                for k in range(NIF):
                    rhs = xq[:, k, :]
                    st = k == 0
                    nc.tensor.matmul(kp, wkT_sb[:, k, :], rhs,
                                     start=st, stop=False)
                    nc.tensor.matmul(vp, wvT_sb[:, k, :], rhs,
                                     start=st, stop=False)
                rwxk = lora_rw(pA, pAd, hk, 512, "k")
                hpk = pA.tile([64, 512], BF16, name="hpk", tag="hp", bufs=2)
                nc.vector.tensor_tensor(hpk, hk[0:64, :], rwxk, AluOpType.mult)
                nc.tensor.matmul(kp, bk_sb[:, 0:64], hpk, start=False, stop=True)
                rwxv = lora_rw(pA, pAd, hv, 512, "v")
                hpv = pA.tile([64, 512], BF16, name="hpv", tag="hp", bufs=2)
                nc.vector.tensor_tensor(hpv, hv[0:64, :], rwxv, AluOpType.mult)
                nc.tensor.matmul(vp, bv_sb[:, 0:64], hpv, start=False, stop=True)
                nc.vector.tensor_copy(k_pre[:, 0, tsl], kp[0:32, :])
                nc.vector.tensor_copy(k_pre[:, 1, tsl], kp[32:64, :])
                nc.vector.tensor_copy(vT_sb[:, tsl], vp)

                # ---- per-tb RoPE + head rearrange + token-major v ----
                tmp = pA.tile([64, 512], F32, name="tmp", tag="tmp", bufs=2)
                tm2 = pA.tile([64, 512], F32, name="tm2", tag="tm2", bufs=2)
                for page in range(2):
                    qe = q_pre[:, 0, page, tsl]
                    qo = q_pre[:, 1, page, tsl]
                    cst = cs_sb[:, tsl]
                    snt = sn_sb[:, tsl]
                    nc.vector.tensor_tensor(tmp, qe, cst, AluOpType.mult)
                    nc.vector.tensor_tensor(tm2, qo, snt, AluOpType.mult)
                    nc.vector.tensor_tensor(qrot[:, 0, page, tsl], tmp, tm2,
                                            AluOpType.subtract)
                    nc.vector.tensor_tensor(tmp, qe, snt, AluOpType.mult)
                    nc.vector.tensor_tensor(tm2, qo, cst, AluOpType.mult)
                    nc.vector.tensor_tensor(qrot[:, 1, page, tsl], tmp, tm2,
                                            AluOpType.add)
                ke, ko = k_pre[:, 0, tsl], k_pre[:, 1, tsl]
                te, to = tmp[0:32, :], tm2[0:32, :]
                nc.vector.tensor_tensor(te, ke, cs_sb[0:32, tsl],
                                        AluOpType.mult)
                nc.vector.tensor_tensor(to, ko, sn_sb[0:32, tsl],
                                        AluOpType.mult)
                nc.vector.tensor_tensor(krot[:, 0, tsl], te, to,
                                        AluOpType.subtract)
                nc.vector.tensor_tensor(te, ke, sn_sb[0:32, tsl],
                                        AluOpType.mult)
                nc.vector.tensor_tensor(to, ko, cs_sb[0:32, tsl],
                                        AluOpType.mult)
                nc.vector.tensor_tensor(krot[:, 1, tsl], te, to,
                                        AluOpType.add)
                for h in range(QH):
                    page, i = h // 2, h % 2
                    nc.scalar.dma_start(
                        out=qh_sb[64 * i:64 * i + 32, page, tsl],
                        in_=qrot[32 * i:32 * i + 32, 0, page, tsl])
                    nc.scalar.dma_start(
                        out=qh_sb[64 * i + 32:64 * i + 64, page, tsl],
                        in_=qrot[32 * i:32 * i + 32, 1, page, tsl])
                for half in range(2):
                    nc.scalar.dma_start(
                        out=kh_sb[64 * half:64 * half + 32, tsl],
                        in_=krot[:, 0, tsl])
                    nc.scalar.dma_start(
                        out=kh_sb[64 * half + 32:64 * half + 64, tsl],
                        in_=krot[:, 1, tsl])
                for j in range(4):
                    kt = 4 * tb + j
                    v_ps = ps.tile([128, 64], BF16, name="v_ps", tag="b_tp")
                    nc.tensor.transpose(v_ps,
                                        vT_sb[:, 128 * kt:128 * kt + 128],
                                        ident_b[0:64, 0:64])
                    nc.vector.tensor_copy(vtok[:, kt, 0:64], v_ps)
                    nc.vector.memset(vtok[:, kt, 64:65], 1.0)

        # prefetch the full output-projection weight during attention
        wo_ctx = tc.tile_pool(name="wo_pool", bufs=4)
        wo_pool = wo_ctx.__enter__()
        wo_tiles = []
        for ob in range(4):
            osl = slice(ob * 512, (ob + 1) * 512)
            wo_sb = wo_pool.tile([128, NIF, 512], BF16, name="wo_sb",
                                 tag="wo", bufs=4)
            nc.sync.dma_start(
                out=wo_sb,
                in_=woT.rearrange("(n p) f -> p n f", p=128)[:, :, osl])
            wo_tiles.append(wo_sb)

        # ================= Phase C: attention =================
        if BUILD_MODE == "A":
            zt = persist.tile([128, 512], F32, name="zt")
            nc.vector.memset(zt, 0.0)
            for tt in range(2):
                for ob in range(4):
                    nc.sync.dma_start(
                        out=y[128 * tt:128 * tt + 128,
                              512 * ob:512 * ob + 512], in_=zt)
            return
        with tc.tile_pool(name="pC", bufs=1) as pC, \
                tc.tile_pool(name="pCd", bufs=2, space="DRAM") as pCd:
            SC_TAGS = ["b_q0", "b_q1", "b_hq", "b_tp"]
            OUT_TAGS = ["b_kp", "b_hk", "b_hv", "b_vp"]
            for qb in range(NQB):
                qsl = slice(qb * 512, (qb + 1) * 512)
                active = [kt for kt in range(NKT) if mask_cls[kt, qb] != M_SKIP]
                assert active, f"fully masked query block qb={qb}"
                outps = [ps.tile([65, 512], F32, name="outp%d" % h,
                                 tag=OUT_TAGS[h]) for h in range(QH)]
                for kt in active:
                    c = mask_cls[kt, qb]
                    mt = None
                    if c == M_ADD:
                        mt = pC.tile([128, 512], BF16, name="mt",
                                     tag="mt", bufs=4)
                        nc.gpsimd.dma_start(
                            out=mt,
                            in_=maskT[128 * kt:128 * kt + 128, qsl])
                    ksl = slice(128 * kt, 128 * kt + 128)
                    scs = []
                    for h in range(QH):
                        page, i = h // 2, h % 2
                        sc = ps.tile([128, 512], F32, name="sc%d" % h,
                                     tag=SC_TAGS[h])
                        # heads with i=1 run in array rows 64..127,
                        # concurrent with the i=0 head of the same page
                        nc.tensor.matmul(sc,
                                         kh_sb[64 * i:64 * i + 64, ksl],
                                         qh_sb[64 * i:64 * i + 64, page, qsl],
                                         start=True, stop=True,
                                         tile_position=(64 * i, 0))
                        scs.append(sc)
                    for h in range(QH):
                        sc = scs[h]
                        if mt is not None:
                            nc.vector.tensor_tensor(sc, sc, mt, AluOpType.add)
                        pr = pC.tile([128, 512], BF16, name="pr",
                                     tag="pr", bufs=10)
                        nc.scalar.activation(pr, sc, AF.Exp)
                        nc.tensor.matmul(outps[h], vtok[:, kt, :], pr,
                                         start=(kt == active[0]),
                                         stop=(kt == active[-1]))
                # ship unnormalized sums + denominators through the A2A;
                # normalization happens post-reshard with one cheap recip
                for h in range(QH):
                    on65 = pC.tile([65, 512], BF16, name="on65", tag="on65",
                                   bufs=4)
                    nc.vector.tensor_copy(on65, outps[h])
                    for half in range(2):
                        hsl = slice(256 * half, 256 * half + 256)
                        nc.gpsimd.dma_start(
                            out=cc_in[2 * qb + half, 64 * h:64 * h + 64, :],
                            in_=on65[0:64, hsl])
                        nc.gpsimd.dma_start(
                            out=cc_in[2 * qb + half, QF + h, :],
                            in_=on65[64:65, hsl])

        # ================= Phase D: AllToAll + o-proj =================
        if BUILD_MODE == "C":
            zt = persist.tile([128, 512], F32, name="zt")
            nc.vector.memset(zt, 0.0)
            for tt in range(2):
                for ob in range(4):
                    nc.sync.dma_start(
                        out=y[128 * tt:128 * tt + 128,
                              512 * ob:512 * ob + 512], in_=zt)
            return
        nc.gpsimd.collective_compute(
            "AllToAll",
            AluOpType.bypass,
            ins=[cc_in[:]],
            outs=[cc_out[:]],
            replica_groups=[list(range(NCORES))],
        )

        if BUILD_MODE == "CC":
            zt = persist.tile([128, 512], F32, name="zt")
            nc.vector.memset(zt, 0.0)
            for tt in range(2):
                for ob in range(4):
                    nc.sync.dma_start(
                        out=y[128 * tt:128 * tt + 128,
                              512 * ob:512 * ob + 512], in_=zt)
            return
        with tc.tile_pool(name="pD", bufs=1) as pD, \
                tc.tile_pool(name="pDd", bufs=1, space="DRAM") as pDd:
            g_v = g_sb.rearrange("p (c n) t -> p c n t", n=2)
            for n in range(2):
                nc.sync.dma_start(
                    out=g_v[:, :, n, :],
                    in_=cc_out[:, 128 * n:128 * n + 128, :]
                        .rearrange("c p t -> p c t"))
            den_all = pD.tile([32, TSH], BF16, name="den_all")
            for cb in range(NCORES):
                nc.sync.dma_start(
                    out=den_all[QH * cb:QH * cb + QH, :],
                    in_=cc_out[cb, QF:QF + QH, :])
            rec32 = pD.tile([32, TSH], F32, name="rec32")
            nc.vector.reciprocal(rec32, den_all)
            g_n = pD.tile([128, NIF, TSH], BF16, name="g_n")
            for k in range(NIF):
                rb_ps = ps.tile([128, TSH], F32, name="rb_ps",
                                tag="b_q0" if k % 2 == 0 else "b_q1")
                nc.tensor.matmul(rb_ps, sel_sb[:, 128 * k:128 * k + 128],
                                 rec32, start=True, stop=True)
                nc.vector.tensor_tensor(g_n[:, k, :], g_sb[:, k, :], rb_ps,
                                        AluOpType.mult)
            ho = ps.tile([72, TSH], F32, name="ho", tag="b_hq")
            for k in range(NIF):
                nc.tensor.matmul(ho, a_sb["o"][:, k, :], g_n[:, k, :],
                                 start=(k == 0), stop=(k == NIF - 1))
            rwxo = lora_rw(pD, pDd, ho, TSH, "o")
            hpo = pD.tile([64, TSH], BF16, name="hpo")
            nc.vector.tensor_tensor(hpo, ho[0:64, :], rwxo, AluOpType.mult)

            for ob in range(4):
                osl = slice(ob * 512, (ob + 1) * 512)
                wo_sb = wo_tiles[ob]
                for tt in range(2):
                    yp = ps.tile([128, 512], F32, name="yp",
                                 tag="b_vp" if (2 * ob + tt) % 2 == 0
                                 else "b_hv")
                    for k in range(NIF):
                        nc.tensor.matmul(yp, g_n[:, k, 128 * tt:128 * tt + 128],
                                         wo_sb[:, k, :], start=(k == 0),
                                         stop=False)
                    nc.tensor.matmul(yp, hpo[:, 128 * tt:128 * tt + 128],
                                     bo_sb[:, osl], start=False, stop=True)
                    yt = pD.tile([128, 512], F32, name="yt", tag="yt", bufs=2)
                    nc.vector.tensor_copy(yt, yp)
                    nc.sync.dma_start(out=y[128 * tt:128 * tt + 128, osl],
                                      in_=yt)
        wo_ctx.__exit__(None, None, None)


# ======================= host side =======================

_CACHE = {}


def _prep_inputs(x, mask, freqs_cos, freqs_sin, wq, wk, wv, wo,
                 lq_router, lq_A, lq_B, lk_router, lk_A, lk_B,
                 lv_router, lv_A, lv_B, lo_router, lo_A, lo_B):
    scale = 1.0 / np.sqrt(HD)
    x = _f32(np.asarray(x)).reshape(S, D)
    maskf = _f32(np.asarray(mask)).reshape(S, S)
    maskT = np.maximum(maskf, MASK_NEG).T.copy()
    mask_cls = classify_mask(maskT)

    xT = _bf(x.T)
    cs2 = _bf(np.tile(_f32(freqs_cos).T, (2, 1)))      # [64, S]
    sn2 = _bf(np.tile(_f32(freqs_sin).T, (2, 1)))
    woT = _bf(_f32(wo).T)
    maskT_bf = _bf(maskT)
    ao_p = _bf(_lora_pack(_f32(lo_A), _f32(lo_router)))
    bo_f = _bf(_b_flat(_f32(lo_B), SCALING))

    sel = np.zeros((H, NIF * 128), dtype=np.float32)
    for k in range(NIF):
        for p in range(128):
            sel[2 * k + p // 64, 128 * k + p] = 1.0
    shared = dict(xT=xT, cs2=cs2, sn2=sn2, woT=woT, maskT=maskT_bf,
                  ao=ao_p, bo=bo_f, sel=sel)

    aq_p = _bf(_lora_pack(_f32(lq_A), _f32(lq_router)))
    ak_p = _bf(_lora_pack(_f32(lk_A), _f32(lk_router)))
    av_p = _bf(_lora_pack(_f32(lv_A), _f32(lv_router)))

    wqf, wkf, wvf = _f32(wq), _f32(wk), _f32(wv)
    lqB, lkB, lvB = _f32(lq_B), _f32(lk_B), _f32(lv_B)

    in_maps = []
    for c in range(NCORES):
        wq_c = wqf[c * QF:(c + 1) * QF][IDX_Q] * scale
        wk_c = wkf[c * KF:(c + 1) * KF][IDX_K]
        wv_c = wvf[c * KF:(c + 1) * KF]
        bq_c = _b_flat(lqB[:, c * QF:(c + 1) * QF, :][:, IDX_Q, :],
                       SCALING * scale)
        bk_c = _b_flat(lkB[:, c * KF:(c + 1) * KF, :][:, IDX_K, :], SCALING)
        bv_c = _b_flat(lvB[:, c * KF:(c + 1) * KF, :], SCALING)
        m = dict(shared)
        m.update(wqT=_bf(wq_c.T), wkT=_bf(wk_c.T), wvT=_bf(wv_c.T),
                 aq=aq_p, ak=ak_p, av=av_p,
                 bq=_bf(bq_c), bk=_bf(bk_c), bv=_bf(bv_c))
        in_maps.append(m)
    return in_maps, mask_cls


def get_graph(mask_cls):
    key = mask_cls.tobytes()
    if key not in _CACHE:
        _CACHE[key] = build(mask_cls)
    return _CACHE[key]


def kernel(x, start_pos, mask, freqs_cos, freqs_sin, wq, wk, wv, wo,
           lq_router, lq_A, lq_B, lk_router, lk_A, lk_B,
           lv_router, lv_A, lv_B, lo_router, lo_A, lo_B,
           _trace=False):
    from concourse.bass_utils import run_bass_kernel_spmd
    in_maps, mask_cls = _prep_inputs(
        x, mask, freqs_cos, freqs_sin, wq, wk, wv, wo,
        lq_router, lq_A, lq_B, lk_router, lk_A, lk_B,
        lv_router, lv_A, lv_B, lo_router, lo_A, lo_B)
    nc = get_graph(mask_cls)
    res = run_bass_kernel_spmd(nc, in_maps, list(range(NCORES)), trace=_trace)
    out = np.concatenate([res.results[c]["y"] for c in range(NCORES)], axis=0)
    out = out.reshape(B, S, H * HD).astype(np.float32)
    if _trace:
        return out, res
    return out



# revision 39
# speedup vs baseline: 1.5988x; 1.5988x over previous
"""Trainium2 Bass kernel for MoE-LoRA GQA attention (nn_Attention_57389353009692).

Strategy (8 NeuronCores, one SPMD launch), v2:
  - Tensor-parallel over heads: core c owns q-heads 4c..4c+3 and kv-head c.
  - Phase A (per 512-token block): packed QKV projections (q0|q1|kv plus
    LoRA-A/router packs, 5 PSUM accumulators), router softmax entirely
    on-chip (exp -> ones-matmul sum -> reciprocal -> selector-matmul
    partition broadcast; no transposes, no DRAM roundtrip), fp16 RoPE on
    full 128-partition tiles, head rearrange via small SBUF DMAs.
  - Phase C: flash-style attention at (128 key x 256 query) granularity.
    All 4 q-heads share one kv head (GQA), so one score matmul covers two
    heads (moving = 2x256 queries). Scores in fp32 PSUM (ping-ponged
    2-bank tiles), one fused exp per half-block with a constant bias
    2^-8 folded in (cancels in normalization), mask-add only on partial
    diagonal half-blocks using deduped mask patterns. Unnormalized AV sums
    + denominators accumulate in one 4-bank PSUM tile per 512-query block.
  - Two AllToAlls (tokens 0-1023 after qb1, 1024-2047 after qb3) reshard
    head-major -> token-major; the first overlaps with qb2/qb3 attention.
  - Phase D per 128-token half: normalize by denominators (selector-matmul
    broadcast), o-projection against the full wo (prefetched during phase
    C) + o-LoRA, direct PSUM->DRAM output.

Numerics: fp16 operands for all matmuls (accumulation fp32 in PSUM),
softmaxes in fp32 without max-subtraction (scores are O(1); the mask is
clamped to -60000 so fp16/exp underflow to exactly 0). Scale 1/sqrt(64)
folded into wq and the q-LoRA B on host.
"""

import os
import sys

for _p in ("/opt/trn_rl_repo", "/root/.axon_site/_ro/trn_rl_repo"):
    if _p not in sys.path:
        sys.path.insert(0, _p)

KDBG = os.environ.get("KDBG", "")

import numpy as np
import ml_dtypes

import concourse.bass as bass
import concourse.tile as tile
from concourse import bacc, mybir
from concourse.masks import make_identity
from concourse.alu_op_type import AluOpType

F32 = mybir.dt.float32
FP16 = mybir.dt.float16
AF = mybir.ActivationFunctionType
FP16NP = np.float16

B, S, D = 1, 2048, 2048
H, KVH, HD = 32, 8, 64
R, E = 8, 8
SCALING = 32.0 / 8.0
NCORES = 8
QH = H // NCORES          # 4 q heads per core
QF = QH * HD              # 256 q feats per core
NKT = S // 128            # 16 key tiles
NQB = S // 512            # 4 query blocks
NIF = D // 128            # 16 contraction tiles
TD = S // NCORES // 2     # 128 tokens per core per A2A half

MASK_NEG = -60000.0
EXP_BIAS = -5.545177444479562  # -8*ln2: pr scaled 2^-8, cancels in norm

M_SKIP, M_FREE = -2, -1  # cls >= 0 -> mask pattern index


def _build_perm():
    idx_q = np.zeros(QF, dtype=np.int64)
    for f in range(QF):
        blk, w = divmod(f, 128)
        h, j = divmod(w, 32)
        idx_q[f] = 64 * h + 2 * j + blk
    idx_k = np.zeros(HD, dtype=np.int64)
    for w in range(HD):
        idx_k[w] = 2 * w if w < 32 else 2 * (w - 32) + 1
    return idx_q, idx_k


IDX_Q, IDX_K = _build_perm()


def _fp16(x):
    return np.ascontiguousarray(np.asarray(x, dtype=np.float32)).astype(FP16NP)


def _f32(x):
    return np.ascontiguousarray(x, dtype=np.float32)


def _a_pack(A):
    """[E,R,D] -> [D, 64] columns ordered r*8+e."""
    return np.transpose(A, (1, 0, 2)).reshape(E * R, -1).T


def _b_flat(Bw, scale):
    """[E, OF, R] -> [64, OF] with row r*8+e."""
    return np.transpose(Bw, (2, 0, 1)).reshape(E * R, -1) * scale


def classify_mask(maskT):
    """maskT: [S(k), S(q)] clamped fp32.

    Returns cls[NKT, NQB, 2] with M_SKIP / M_FREE / pattern-index, and the
    deduped pattern list [[128, 256] fp32, ...].
    """
    cls = np.full((NKT, NQB, 2), M_SKIP, dtype=np.int64)
    patterns = []
    keys = {}
    for qb in range(NQB):
        for kt in range(NKT):
            rows = maskT[kt * 128:(kt + 1) * 128]
            for hf in range(2):
                blk = rows[:, qb * 512 + hf * 256: qb * 512 + hf * 256 + 256]
                if np.all(blk <= MASK_NEG * 0.5):
                    cls[kt, qb, hf] = M_SKIP
                elif np.all(blk == 0.0):
                    cls[kt, qb, hf] = M_FREE
                else:
                    kb = blk.astype(np.float32).tobytes()
                    if kb not in keys:
                        keys[kb] = len(patterns)
                        patterns.append(blk.astype(np.float32))
                    cls[kt, qb, hf] = keys[kb]
    return cls, patterns


def build(cls, n_pat):
    nc = bacc.Bacc(None, target_bir_lowering=False)

    xT = nc.declare_dram_parameter("xT", [D, S], FP16, isOutput=False)
    wqT = nc.declare_dram_parameter("wqT", [D, QF], FP16, isOutput=False)
    wkvT = nc.declare_dram_parameter("wkvT", [D, 128], FP16, isOutput=False)
    aqk = nc.declare_dram_parameter("aqk", [D, 128], FP16, isOutput=False)
    avr = nc.declare_dram_parameter("avr", [D, 88], FP16, isOutput=False)
    aob = nc.declare_dram_parameter("aob", [D, 72], FP16, isOutput=False)
    bq = nc.declare_dram_parameter("bq", [64, QF], FP16, isOutput=False)
    bkv = nc.declare_dram_parameter("bkv", [128, 64], FP16, isOutput=False)
    bv = nc.declare_dram_parameter("bv", [64, 64], FP16, isOutput=False)
    bo = nc.declare_dram_parameter("bo", [64, D], FP16, isOutput=False)
    woT = nc.declare_dram_parameter("woT", [D, D], FP16, isOutput=False)
    cs4 = nc.declare_dram_parameter("cs4", [128, S], FP16, isOutput=False)
    sn4 = nc.declare_dram_parameter("sn4", [128, S], FP16, isOutput=False)
    sels = nc.declare_dram_parameter("sels", [24, 516], FP16, isOutput=False)
    selk = nc.declare_dram_parameter("selk", [32, NIF * 128], FP16,
                                     isOutput=False)
    masku = nc.declare_dram_parameter("masku", [128, max(n_pat, 1) * 256],
                                      FP16, isOutput=False)
    y = nc.declare_dram_parameter("y", [2 * TD, D], F32, isOutput=True)

    cc_in = [nc.dram_tensor(f"cc{i}_in", [NCORES, QF + QH, TD], FP16)
             for i in range(2)]
    cc_out = [nc.dram_tensor(f"cc{i}_out", [NCORES, QF + QH, TD], FP16)
              for i in range(2)]

    with tile.TileContext(nc) as tc:
        _emit(nc, tc, locals(), cls, n_pat)
    nc.finalize()
    return nc


def _emit(nc, tc, t, cls, n_pat):
    import contextlib
    ctx = contextlib.ExitStack()
    with ctx:
        persist = ctx.enter_context(tc.tile_pool(name="persist", bufs=1))

        # ---- persistent weights ----
        wqT_sb = persist.tile([128, NIF, QF], FP16)
        nc.scalar.dma_start(out=wqT_sb,
                            in_=t["wqT"].rearrange("(n p) f -> p n f", p=128))
        wkvT_sb = persist.tile([128, NIF, 128], FP16)
        nc.scalar.dma_start(out=wkvT_sb,
                            in_=t["wkvT"].rearrange("(n p) f -> p n f", p=128))
        aqk_sb = persist.tile([128, NIF, 128], FP16)
        nc.gpsimd.dma_start(out=aqk_sb,
                            in_=t["aqk"].rearrange("(n p) f -> p n f", p=128))
        avr_sb = persist.tile([128, NIF, 88], FP16)
        nc.gpsimd.dma_start(out=avr_sb,
                            in_=t["avr"].rearrange("(n p) f -> p n f", p=128))
        aob_sb = persist.tile([128, NIF, 72], FP16)
        nc.gpsimd.dma_start(out=aob_sb,
                            in_=t["aob"].rearrange("(n p) f -> p n f", p=128))
        bq_sb = persist.tile([64, QF], FP16)
        nc.gpsimd.dma_start(out=bq_sb, in_=t["bq"][:])
        bkv_sb = persist.tile([128, 64], FP16)
        nc.gpsimd.dma_start(out=bkv_sb, in_=t["bkv"][:])
        bv_sb = persist.tile([64, 64], FP16)
        nc.gpsimd.dma_start(out=bv_sb, in_=t["bv"][:])
        bo_sb = persist.tile([64, D], FP16)
        nc.gpsimd.dma_start(out=bo_sb, in_=t["bo"][:])
        cs4_sb = persist.tile([128, S], FP16)
        nc.gpsimd.dma_start(out=cs4_sb, in_=t["cs4"][:])
        sn4_sb = persist.tile([128, S], FP16)
        nc.gpsimd.dma_start(out=sn4_sb, in_=t["sn4"][:])
        # sels columns: [0:128 sel_qk | 128:192 sel_v | 192:320 sel3_qk(3r) |
        #               320:384 pad | 384:387 ones_sel.T? -> see host packing]
        sels_sb = persist.tile([24, 516], FP16)
        nc.gpsimd.dma_start(out=sels_sb, in_=t["sels"][:])
        selk_sb = persist.tile([32, NIF, 128], FP16)
        nc.gpsimd.dma_start(
            out=selk_sb, in_=t["selk"].rearrange("h (n f) -> h n f", f=128))
        masku_sb = persist.tile([128, max(n_pat, 1), 256], FP16)
        nc.gpsimd.dma_start(
            out=masku_sb,
            in_=t["masku"].rearrange("p (u f) -> p u f", f=256))

        sel_qk = sels_sb[:, 0:128]
        sel_v = sels_sb[:, 128:192]
        ones_blk = sels_sb[:, 192:216]   # [24, 24] block-diag ones
        sel_o = sels_sb[0:8, 216:280]
        ones8x8 = sels_sb[0:8, 280:288]  # [8, 8] all ones

        ident_h = persist.tile([64, 64], FP16)
        make_identity(nc, ident_h)
        ebias = persist.tile([128, 1], F32)
        nc.vector.memset(ebias, EXP_BIAS)

        qh_sb = persist.tile([64, QH, S], FP16)
        kh_sb = persist.tile([64, S], FP16)
        vT_sb = persist.tile([64, S], FP16)
        vtok = persist.tile([128, NKT, 65], FP16)
        nc.vector.memset(vtok[:, :, 64:65], 1.0)
        woT_sb = persist.tile([128, NIF, D], FP16)  # DMAs issued after A

        # ================= Phase A =================
        with nc.named_scope("phaseA"), \
                tc.tile_pool(name="pA", bufs=1) as pA, \
                tc.tile_pool(name="psA", bufs=1, space="PSUM") as ps:
            for tb in range(4):
                tsl = slice(tb * 512, (tb + 1) * 512)
                xq = pA.tile([128, NIF, 512], FP16, name="xq", tag="xq",
                             bufs=2)
                nc.sync.dma_start(
                    out=xq,
                    in_=t["xT"].rearrange("(n p) s -> p n s", p=128)[:, :, tsl])

                la = ps.tile([128, 512], F32, name="la", tag="b_la")
                lv = ps.tile([88, 512], F32, name="lv", tag="b_lv")
                q0 = ps.tile([128, 512], F32, name="q0", tag="b_q0")
                q1 = ps.tile([128, 512], F32, name="q1", tag="b_q1")
                kv = ps.tile([128, 512], F32, name="kv", tag="b_kv")
                lsc = ps.tile([128, 512], F32, name="lsc", tag="b_lsc")
                rwx = ps.tile([128, 512], F32, name="rwx", tag="b_rwx")
                vps = ps.tile([128, 2, 64], FP16, name="vps", tag="b_vps")

                for k in range(NIF):
                    st, sp = k == 0, k == NIF - 1
                    nc.tensor.matmul(la, aqk_sb[:, k, :], xq[:, k, :],
                                     start=st, stop=sp)
                for k in range(NIF):
                    st, sp = k == 0, k == NIF - 1
                    nc.tensor.matmul(lv, avr_sb[:, k, :], xq[:, k, :],
                                     start=st, stop=sp)
                # router softmax (fp32, no max-subtract; logits lv[64:88])
                ex_h = pA.tile([24, 512], FP16, name="ex_h", tag="ex_h",
                               bufs=2)
                nc.scalar.activation(ex_h, lv[64:88, :], AF.Exp)

                for k in range(NIF):
                    nc.tensor.matmul(q0, wqT_sb[:, k, 0:128], xq[:, k, :],
                                     start=(k == 0), stop=False)
                # per-group softmax sums at 24 partitions -> lsc rows 0:24
                nc.tensor.matmul(lsc[0:24, :], ones_blk, ex_h,
                                 start=True, stop=True)
                rec24 = pA.tile([24, 512], FP16, name="rec24", tag="rec24",
                                bufs=2)
                with nc.allow_low_precision(reason="router softmax recip"):
                    nc.vector.reciprocal(rec24, lsc[0:24, :])
                rw_n = pA.tile([24, 512], FP16, name="rw_n", tag="rw_n",
                               bufs=2)
                nc.vector.tensor_tensor(rw_n, ex_h, rec24, AluOpType.mult)

                for k in range(NIF):
                    nc.tensor.matmul(q1, wqT_sb[:, k, 128:256], xq[:, k, :],
                                     start=(k == 0), stop=False)
                nc.tensor.matmul(rwx, sel_qk, rw_n, start=True, stop=True)
                la_sb = pA.tile([128, 512], FP16, name="la_sb", tag="la_sb",
                                bufs=2)
                nc.vector.tensor_copy(la_sb, la)
                hpqk = pA.tile([128, 512], FP16, name="hpqk", tag="hpqk",
                               bufs=2)
                nc.vector.tensor_tensor(hpqk, la_sb, rwx, AluOpType.mult)

                for k in range(NIF):
                    nc.tensor.matmul(kv, wkvT_sb[:, k, :], xq[:, k, :],
                                     start=(k == 0), stop=False)
                # v-lora combine (reuses the lsc bank after rec24's read)
                nc.tensor.matmul(lsc[0:64, :], sel_v, rw_n,
                                 start=True, stop=True)
                lv_sb = pA.tile([64, 512], FP16, name="lv_sb", tag="lv_sb",
                                bufs=2)
                nc.vector.tensor_copy(lv_sb, lv[0:64, :])
                hpv = pA.tile([64, 512], FP16, name="hpv", tag="hpv", bufs=2)
                nc.vector.tensor_tensor(hpv, lv_sb, lsc[0:64, :],
                                        AluOpType.mult)

                # LoRA-B accumulations
                nc.tensor.matmul(q0, bq_sb[:, 0:128], hpqk[0:64, :],
                                 start=False, stop=True)
                nc.tensor.matmul(q1, bq_sb[:, 128:256], hpqk[0:64, :],
                                 start=False, stop=True)
                nc.tensor.matmul(kv[0:64, :], bkv_sb[64:128, :],
                                 hpqk[64:128, :], start=False, stop=True)
                nc.tensor.matmul(kv[64:128, :], bv_sb, hpv,
                                 start=False, stop=True)

                # ---- RoPE (fp16) ----
                q0c = pA.tile([128, 512], FP16, name="q0c", tag="q0c", bufs=2)
                nc.vector.tensor_copy(q0c, q0)
                q1c = pA.tile([128, 512], FP16, name="q1c", tag="q1c", bufs=2)
                nc.scalar.activation(q1c, q1, AF.Copy)
                csl = cs4_sb[:, tsl]
                snl = sn4_sb[:, tsl]
                t1 = pA.tile([128, 512], FP16, name="t1", tag="t1", bufs=2)
                t2 = pA.tile([128, 512], FP16, name="t2", tag="t2", bufs=2)
                qre = pA.tile([128, 512], FP16, name="qre", tag="qre", bufs=2)
                qro = pA.tile([128, 512], FP16, name="qro", tag="qro", bufs=2)
                nc.vector.tensor_tensor(t1, q0c, csl, AluOpType.mult)
                nc.vector.tensor_tensor(t2, q1c, snl, AluOpType.mult)
                nc.vector.tensor_tensor(qre, t1, t2, AluOpType.subtract)
                nc.vector.tensor_tensor(t1, q0c, snl, AluOpType.mult)
                nc.vector.tensor_tensor(t2, q1c, csl, AluOpType.mult)
                nc.vector.tensor_tensor(qro, t1, t2, AluOpType.add)

                kc2 = pA.tile([32, 2, 512], FP16, name="kc2", tag="kc2",
                              bufs=2)
                nc.vector.tensor_copy(kc2[:, 0, :], kv[0:32, :])
                nc.vector.tensor_copy(kc2[:, 1, :], kv[32:64, :])
                nc.scalar.activation(vT_sb[:, tsl], kv[64:128, :], AF.Copy)
                csl32 = cs4_sb[0:32, tsl]
                snl32 = sn4_sb[0:32, tsl]
                tk1 = pA.tile([32, 512], FP16, name="tk1", tag="tk1", bufs=2)
                tk2 = pA.tile([32, 512], FP16, name="tk2", tag="tk2", bufs=2)
                kho = pA.tile([32, 512], FP16, name="kho", tag="kho", bufs=2)
                nc.vector.tensor_tensor(tk1, kc2[:, 0, :], csl32,
                                        AluOpType.mult)
                nc.vector.tensor_tensor(tk2, kc2[:, 1, :], snl32,
                                        AluOpType.mult)
                nc.vector.tensor_tensor(kh_sb[0:32, tsl], tk1, tk2,
                                        AluOpType.subtract)
                nc.vector.tensor_tensor(tk1, kc2[:, 0, :], snl32,
                                        AluOpType.mult)
                nc.vector.tensor_tensor(tk2, kc2[:, 1, :], csl32,
                                        AluOpType.mult)
                nc.vector.tensor_tensor(kho, tk1, tk2, AluOpType.add)
                nc.vector.tensor_copy(kh_sb[32:64, tsl], kho)

                for h in range(QH):
                    nc.sync.dma_start(
                        out=qh_sb[0:32, h, tsl],
                        in_=qre[32 * h:32 * h + 32, :])
                    nc.sync.dma_start(
                        out=qh_sb[32:64, h, tsl],
                        in_=qro[32 * h:32 * h + 32, :])

                # token-major v
                for j in range(4):
                    kt = 4 * tb + j
                    nc.tensor.transpose(
                        vps[:, j % 2, :], vT_sb[:, 128 * kt:128 * kt + 128],
                        ident_h)
                    nc.vector.tensor_copy(vtok[:, kt, 0:64], vps[:, j % 2, :])

        # prefetch wo during attention
        for j, eng in enumerate((nc.sync, nc.scalar, nc.gpsimd, nc.sync)):
            eng.dma_start(
                out=woT_sb[:, :, 512 * j:512 * j + 512],
                in_=t["woT"].rearrange("(n p) f -> p n f", p=128)
                [:, :, 512 * j:512 * j + 512])

        if KDBG == "qkv":
            with tc.tile_pool(name="pX", bufs=1) as pX:
                for r, src in enumerate((qh_sb[:, 0, :], kh_sb, vT_sb,
                                         qh_sb[:, 1, :])):
                    ytd = pX.tile([64, S], F32, name="ytd", tag="ytd", bufs=2)
                    nc.vector.tensor_copy(ytd, src)
                    nc.sync.dma_start(out=t["y"][64 * r:64 * r + 64, :],
                                      in_=ytd)
            return

        # ================= Phase C =================
        sc_i = 0
        with nc.named_scope("phaseC"), \
                tc.tile_pool(name="pC", bufs=1) as pC, \
                tc.tile_pool(name="psC", bufs=1, space="PSUM") as ps:
            for qb in range(NQB):
                outp = ps.tile([65, 2, 2, 2, 256], F32, name="outp",
                               tag="outp")
                first = {}
                last = {}
                for hf in range(2):
                    act = [kt for kt in range(NKT)
                           if cls[kt, qb, hf] != M_SKIP]
                    first[hf], last[hf] = act[0], act[-1]
                for kt in range(4 * qb + 4):
                    ksl = slice(128 * kt, 128 * kt + 128)
                    for hf in range(2):
                        c = cls[kt, qb, hf]
                        if c == M_SKIP:
                            continue
                        qsl = slice(512 * qb + 256 * hf,
                                    512 * qb + 256 * hf + 256)
                        sc = ps.tile([128, 2, 2, 256], F32, name="sc",
                                     tag="b_sc%d" % (sc_i % 2))
                        sc_i += 1
                        for p in range(2):
                            nc.tensor.matmul(
                                sc[:, p, :, :], kh_sb[:, ksl],
                                qh_sb[:, 2 * p:2 * p + 2, qsl],
                                start=True, stop=True)
                        if c >= 0:
                            mt = masku_sb[:, c, :]
                            nc.vector.tensor_tensor(
                                sc, sc,
                                mt.unsqueeze(1).unsqueeze(1)
                                .broadcast_to([128, 2, 2, 256]),
                                AluOpType.add)
                        pr = pC.tile([128, 2, 2, 256], FP16, name="pr",
                                     tag="pr", bufs=3)
                        nc.scalar.activation(pr, sc, AF.Exp, bias=ebias)
                        for p in range(2):
                            nc.tensor.matmul(
                                outp[:, p, hf, :, :], vtok[:, kt, :],
                                pr[:, p, :, :],
                                start=(kt == first[hf]),
                                stop=(kt == last[hf]))
                on65 = pC.tile([65, QH, 512], FP16, name="on65", tag="on65",
                               bufs=2)
                for hf in range(2):
                    nc.vector.tensor_copy(
                        on65[:, :, 256 * hf:256 * hf + 256]
                        .rearrange("P (a i) t -> P a i t", i=2),
                        outp[:, :, hf, :, :])
                if KDBG == "att":
                    if qb == 0:
                        ytd = pC.tile([65, QH * 512], F32, name="ytd")
                        nc.vector.tensor_copy(
                            ytd, on65.rearrange("P h t -> P (h t)"))
                        nc.sync.dma_start(out=t["y"][0:65, :], in_=ytd)
                        ytd2 = pC.tile([128, NKT * 65], F32, name="ytd2")
                        nc.vector.tensor_copy(
                            ytd2, vtok.rearrange("p k c -> p (k c)"))
                        nc.sync.dma_start(
                            out=t["y"][128:256, 0:NKT * 65], in_=ytd2)
                    continue
                cci = t["cc_in"][qb // 2]
                for j in range(4):
                    d = 4 * (qb % 2) + j
                    tj = slice(128 * j, 128 * j + 128)
                    nc.gpsimd.dma_start(
                        out=cci[d, 0:QF, :].rearrange("(h p) t -> p h t",
                                                      p=64),
                        in_=on65[0:64, :, tj])
                    for h in range(QH):
                        nc.gpsimd.dma_start(
                            out=cci[d, QF + h:QF + h + 1, :],
                            in_=on65[64:65, h, tj])
                if qb % 2 == 1 and KDBG != "att":
                    i = qb // 2
                    nc.gpsimd.collective_compute(
                        "AllToAll", AluOpType.bypass,
                        ins=[t["cc_in"][i][:]],
                        outs=[t["cc_out"][i][:]],
                        replica_groups=[list(range(NCORES))],
                    )

        if KDBG == "att":
            return
        # ================= Phase D =================
        with nc.named_scope("phaseD"), \
                tc.tile_pool(name="pD", bufs=1) as pD, \
                tc.tile_pool(name="psD", bufs=1, space="PSUM") as ps:
            for i in range(2):
                cco = t["cc_out"][i]
                g = pD.tile([128, NIF, TD], FP16, name="g", tag="g%d" % i)
                for j in range(2):
                    nc.sync.dma_start(
                        out=g.rearrange("p (c j) t -> p c j t", j=2)
                        [:, :, j, :],
                        in_=cco[:, 128 * j:128 * j + 128, :]
                        .rearrange("c p t -> p c t"))
                den = pD.tile([32, TD], FP16, name="den", tag="den", bufs=2)
                for cb in range(NCORES):
                    nc.sync.dma_start(
                        out=den[QH * cb:QH * cb + QH, :],
                        in_=cco[cb, QF:QF + QH, :])
                if KDBG == "gd":
                    gtd = pD.tile([128, NIF * TD], F32, name="gtd",
                                  tag="gtd", bufs=2)
                    nc.vector.tensor_copy(
                        gtd, g.rearrange("p k t -> p (k t)"))
                    if i == 0:
                        nc.sync.dma_start(out=t["y"][0:128, :], in_=gtd)
                    dtd = pD.tile([32, TD], F32, name="dtd", tag="dtd",
                                  bufs=2)
                    nc.vector.tensor_copy(dtd, den)
                    nc.sync.dma_start(
                        out=t["y"][128 + 32 * i:160 + 32 * i, 0:TD], in_=dtd)
                    continue
                rec = pD.tile([32, TD], FP16, name="rec", tag="rec", bufs=2)
                with nc.allow_low_precision(reason="attn denom recip"):
                    nc.vector.reciprocal(rec, den)
                for k in range(NIF):
                    rb = ps.tile([128, TD], F32, name="rb",
                                 tag="b_rb%d" % (k % 2))
                    nc.tensor.matmul(rb, selk_sb[:, k, :], rec,
                                     start=True, stop=True)
                    nc.vector.tensor_tensor(g[:, k, :], g[:, k, :], rb,
                                            AluOpType.mult)
                ho = ps.tile([128, TD], F32, name="ho", tag="b_ho")
                for k in range(NIF):
                    nc.tensor.matmul(ho[0:72, :], aob_sb[:, k, :], g[:, k, :],
                                     start=(k == 0), stop=(k == NIF - 1))
                ex_o = pD.tile([8, TD], FP16, name="ex_o", tag="ex_o", bufs=2)
                nc.scalar.activation(ex_o, ho[64:72, :], AF.Exp)
                sm_o = ps.tile([128, TD], F32, name="sm_o", tag="b_rb0")
                nc.tensor.matmul(sm_o[0:8, :], ones8x8, ex_o,
                                 start=True, stop=True)
                rec_o = pD.tile([8, TD], FP16, name="rec_o", tag="rec_o",
                                bufs=2)
                with nc.allow_low_precision(reason="o router recip"):
                    nc.vector.reciprocal(rec_o, sm_o[0:8, :])
                rw_o = pD.tile([8, TD], FP16, name="rw_o", tag="rw_o", bufs=2)
                nc.vector.tensor_tensor(rw_o, ex_o, rec_o, AluOpType.mult)
                rt = ps.tile([64, TD], F32, name="rt", tag="b_rt")
                nc.tensor.matmul(rt, sel_o, rw_o, start=True, stop=True)
                ho_sb = pD.tile([64, TD], FP16, name="ho_sb", tag="ho_sb",
                                bufs=2)
                nc.vector.tensor_copy(ho_sb, ho[0:64, :])
                hpo = pD.tile([64, TD], FP16, name="hpo", tag="hpo", bufs=2)
                nc.vector.tensor_tensor(hpo, ho_sb, rt, AluOpType.mult)

                Y = ps.tile([128, 4, 512], F32, name="Y", tag="b_Y")
                for k in range(NIF):
                    for ob in range(4):
                        nc.tensor.matmul(
                            Y[:, ob, :], g[:, k, :],
                            woT_sb[:, k, 512 * ob:512 * ob + 512],
                            start=(k == 0), stop=False)
                for ob in range(4):
                    nc.tensor.matmul(Y[:, ob, :], hpo,
                                     bo_sb[:, 512 * ob:512 * ob + 512],
                                     start=False, stop=True)
                yt = pD.tile([128, 4, 512], F32, name="yt", tag="yt", bufs=2)
                nc.vector.tensor_copy(yt[:, 0:2, :], Y[:, 0:2, :])
                nc.scalar.activation(yt[:, 2:4, :], Y[:, 2:4, :], AF.Copy)
                nc.sync.dma_start(
                    out=t["y"][TD * i:TD * i + TD, :],
                    in_=yt.rearrange("p a f -> p (a f)"))


# ======================= host side =======================

_CACHE = {}


def _prep_inputs(x, mask, freqs_cos, freqs_sin, wq, wk, wv, wo,
                 lq_router, lq_A, lq_B, lk_router, lk_A, lk_B,
                 lv_router, lv_A, lv_B, lo_router, lo_A, lo_B):
    scale = 1.0 / np.sqrt(HD)
    x = _f32(np.asarray(x)).reshape(S, D)
    maskf = _f32(np.asarray(mask)).reshape(S, S)
    maskT = np.maximum(maskf, MASK_NEG).T.copy()
    cls, patterns = classify_mask(maskT)
    n_pat = len(patterns)
    if n_pat:
        masku = np.stack(patterns, axis=1).reshape(128, n_pat * 256)
    else:
        masku = np.zeros((128, 256), dtype=np.float32)

    cos = _f32(freqs_cos)  # [S, 32]
    sin = _f32(freqs_sin)
    cs4 = _fp16(np.tile(cos.T, (4, 1)))
    sn4 = _fp16(np.tile(sin.T, (4, 1)))

    # selector pack [24, 516] (cols: sel_qk 0:128 | sel_v 128:192 |
    #   ones_blk 192:216 | sel_o 216:280 | ones8x8 280:288)
    sels = np.zeros((24, 516), dtype=np.float32)
    for e in range(E):
        for r in range(R):
            sels[e, r * 8 + e] = 1.0                  # sel_qk (q)
            sels[8 + e, 64 + r * 8 + e] = 1.0         # sel_qk (k)
            sels[16 + e, 128 + r * 8 + e] = 1.0       # sel_v
            sels[e, 216 + r * 8 + e] = 1.0            # sel_o
    for j in range(24):
        sels[j, 192 + (j // 8) * 8:192 + (j // 8) * 8 + 8] = 1.0  # ones_blk
    sels[0:8, 280:288] = 1.0                          # ones8x8

    selkm = np.zeros((32, NIF * 128), dtype=np.float32)
    for k in range(NIF):
        for p in range(128):
            selkm[2 * k + p // 64, 128 * k + p] = 1.0

    ao_p = np.concatenate([_a_pack(_f32(lo_A)), _f32(lo_router).T], axis=1)
    shared = dict(xT=_fp16(x.T), cs4=cs4, sn4=sn4, woT=_fp16(_f32(wo).T),
                  masku=_fp16(masku), sels=_fp16(sels), selk=_fp16(selkm),
                  aob=_fp16(ao_p), bo=_fp16(_b_flat(_f32(lo_B), SCALING)))

    aq_p = _a_pack(_f32(lq_A))
    ak_p = _a_pack(_f32(lk_A))
    av_p = _a_pack(_f32(lv_A))
    aqk_p = _fp16(np.concatenate([aq_p, ak_p], axis=1))
    avr_p = _fp16(np.concatenate(
        [av_p, _f32(lq_router).T, _f32(lk_router).T, _f32(lv_router).T],
        axis=1))

    wqf, wkf, wvf = _f32(wq), _f32(wk), _f32(wv)
    lqB, lkB, lvB = _f32(lq_B), _f32(lk_B), _f32(lv_B)

    in_maps = []
    for c in range(NCORES):
        wq_c = wqf[c * QF:(c + 1) * QF][IDX_Q] * scale
        wk_c = wkf[c * HD:(c + 1) * HD][IDX_K]
        wv_c = wvf[c * HD:(c + 1) * HD]
        wkv_c = np.concatenate([wk_c, wv_c], axis=0)
        bq_c = _b_flat(lqB[:, c * QF:(c + 1) * QF, :][:, IDX_Q, :],
                       SCALING * scale)
        bk_c = _b_flat(lkB[:, c * HD:(c + 1) * HD, :][:, IDX_K, :], SCALING)
        bkv_c = np.zeros((128, 64), dtype=np.float32)
        bkv_c[64:128] = bk_c
        bv_c = _b_flat(lvB[:, c * HD:(c + 1) * HD, :], SCALING)
        m = dict(shared)
        m.update(wqT=_fp16(wq_c.T), wkvT=_fp16(wkv_c.T),
                 aqk=aqk_p, avr=avr_p,
                 bq=_fp16(bq_c), bkv=_fp16(bkv_c), bv=_fp16(bv_c))
        in_maps.append(m)
    return in_maps, cls, n_pat


def get_graph(cls, n_pat):
    key = (cls.tobytes(), n_pat, KDBG)
    if key not in _CACHE:
        _CACHE[key] = build(cls, n_pat)
    return _CACHE[key]


def kernel(x, start_pos, mask, freqs_cos, freqs_sin, wq, wk, wv, wo,
           lq_router, lq_A, lq_B, lk_router, lk_A, lk_B,
           lv_router, lv_A, lv_B, lo_router, lo_A, lo_B,
           _trace=False):
    from concourse.bass_utils import run_bass_kernel_spmd
    in_maps, cls, n_pat = _prep_inputs(
        x, mask, freqs_cos, freqs_sin, wq, wk, wv, wo,
        lq_router, lq_A, lq_B, lk_router, lk_A, lk_B,
        lv_router, lv_A, lv_B, lo_router, lo_A, lo_B)
    nc = get_graph(cls, n_pat)
    res = run_bass_kernel_spmd(nc, in_maps, list(range(NCORES)), trace=_trace)
    out = np.empty((S, D), dtype=np.float32)
    for c in range(NCORES):
        yc = res.results[c]["y"]
        out[TD * c:TD * c + TD] = yc[0:TD]
        out[S // 2 + TD * c:S // 2 + TD * c + TD] = yc[TD:2 * TD]
    out = out.reshape(B, S, H * HD)
    if _trace:
        return out, res
    return out


# revision 47
# speedup vs baseline: 1.6044x; 1.0035x over previous
"""Trainium2 Bass kernel for MoE-LoRA GQA attention (nn_Attention_57389353009692).

Strategy (8 NeuronCores, one SPMD launch), v2:
  - Tensor-parallel over heads: core c owns q-heads 4c..4c+3 and kv-head c.
  - Phase A (per 512-token block): packed QKV projections (q0|q1|kv plus
    LoRA-A/router packs, 5 PSUM accumulators), router softmax entirely
    on-chip (exp -> ones-matmul sum -> reciprocal -> selector-matmul
    partition broadcast; no transposes, no DRAM roundtrip), fp16 RoPE on
    full 128-partition tiles, head rearrange via small SBUF DMAs.
  - Phase C: flash-style attention at (128 key x 256 query) granularity.
    All 4 q-heads share one kv head (GQA), so one score matmul covers two
    heads (moving = 2x256 queries). Scores in fp32 PSUM (ping-ponged
    2-bank tiles), one fused exp per half-block with a constant bias
    2^-8 folded in (cancels in normalization), mask-add only on partial
    diagonal half-blocks using deduped mask patterns. Unnormalized AV sums
    + denominators accumulate in one 4-bank PSUM tile per 512-query block.
  - Two AllToAlls (tokens 0-1023 after qb1, 1024-2047 after qb3) reshard
    head-major -> token-major; the first overlaps with qb2/qb3 attention.
  - Phase D per 128-token half: normalize by denominators (selector-matmul
    broadcast), o-projection against the full wo (prefetched during phase
    C) + o-LoRA, direct PSUM->DRAM output.

Numerics: fp16 operands for all matmuls (accumulation fp32 in PSUM),
softmaxes in fp32 without max-subtraction (scores are O(1); the mask is
clamped to -60000 so fp16/exp underflow to exactly 0). Scale 1/sqrt(64)
folded into wq and the q-LoRA B on host.
"""

import os
import sys

for _p in ("/opt/trn_rl_repo", "/root/.axon_site/_ro/trn_rl_repo"):
    if _p not in sys.path:
        sys.path.insert(0, _p)

KDBG = os.environ.get("KDBG", "")

import numpy as np
import ml_dtypes

import concourse.bass as bass
import concourse.tile as tile
from concourse import bacc, mybir
from concourse.masks import make_identity
from concourse.alu_op_type import AluOpType

F32 = mybir.dt.float32
FP16 = mybir.dt.float16
AF = mybir.ActivationFunctionType
FP16NP = np.float16

B, S, D = 1, 2048, 2048
H, KVH, HD = 32, 8, 64
R, E = 8, 8
SCALING = 32.0 / 8.0
NCORES = 8
QH = H // NCORES          # 4 q heads per core
QF = QH * HD              # 256 q feats per core
NKT = S // 128            # 16 key tiles
NQB = S // 512            # 4 query blocks
NIF = D // 128            # 16 contraction tiles
TD = S // NCORES // 2     # 128 tokens per core per A2A half

MASK_NEG = -60000.0
EXP_BIAS = -5.545177444479562  # -8*ln2: pr scaled 2^-8, cancels in norm

M_SKIP, M_FREE = -2, -1  # cls >= 0 -> mask pattern index


def _build_perm():
    idx_q = np.zeros(QF, dtype=np.int64)
    for f in range(QF):
        blk, w = divmod(f, 128)
        h, j = divmod(w, 32)
        idx_q[f] = 64 * h + 2 * j + blk
    idx_k = np.zeros(HD, dtype=np.int64)
    for w in range(HD):
        idx_k[w] = 2 * w if w < 32 else 2 * (w - 32) + 1
    return idx_q, idx_k


IDX_Q, IDX_K = _build_perm()


def _fp16(x):
    return np.ascontiguousarray(np.asarray(x, dtype=np.float32)).astype(FP16NP)


def _f32(x):
    return np.ascontiguousarray(x, dtype=np.float32)


def _a_pack(A):
    """[E,R,D] -> [D, 64] columns ordered r*8+e."""
    return np.transpose(A, (1, 0, 2)).reshape(E * R, -1).T


def _b_flat(Bw, scale):
    """[E, OF, R] -> [64, OF] with row r*8+e."""
    return np.transpose(Bw, (2, 0, 1)).reshape(E * R, -1) * scale


def classify_mask(maskT):
    """maskT: [S(k), S(q)] clamped fp32.

    Returns cls[NKT, NQB, 2] with M_SKIP / M_FREE / pattern-index, and the
    deduped pattern list [[128, 256] fp32, ...].
    """
    cls = np.full((NKT, NQB, 2), M_SKIP, dtype=np.int64)
    patterns = []
    keys = {}
    for qb in range(NQB):
        for kt in range(NKT):
            rows = maskT[kt * 128:(kt + 1) * 128]
            for hf in range(2):
                blk = rows[:, qb * 512 + hf * 256: qb * 512 + hf * 256 + 256]
                if np.all(blk <= MASK_NEG * 0.5):
                    cls[kt, qb, hf] = M_SKIP
                elif np.all(blk == 0.0):
                    cls[kt, qb, hf] = M_FREE
                else:
                    kb = blk.astype(np.float32).tobytes()
                    if kb not in keys:
                        keys[kb] = len(patterns)
                        patterns.append(blk.astype(np.float32))
                    cls[kt, qb, hf] = keys[kb]
    return cls, patterns


def build(cls, n_pat):
    nc = bacc.Bacc(None, target_bir_lowering=False)

    xT = nc.declare_dram_parameter("xT", [D, S], FP16, isOutput=False)
    wqT = nc.declare_dram_parameter("wqT", [D, QF], FP16, isOutput=False)
    wkvT = nc.declare_dram_parameter("wkvT", [D, 128], FP16, isOutput=False)
    aqk = nc.declare_dram_parameter("aqk", [D, 128], FP16, isOutput=False)
    avr = nc.declare_dram_parameter("avr", [D, 88], FP16, isOutput=False)
    aob = nc.declare_dram_parameter("aob", [D, 72], FP16, isOutput=False)
    bq = nc.declare_dram_parameter("bq", [64, QF], FP16, isOutput=False)
    bkv = nc.declare_dram_parameter("bkv", [128, 64], FP16, isOutput=False)
    bv = nc.declare_dram_parameter("bv", [64, 64], FP16, isOutput=False)
    bo = nc.declare_dram_parameter("bo", [64, D], FP16, isOutput=False)
    woT = nc.declare_dram_parameter("woT", [D, D], FP16, isOutput=False)
    cs4 = nc.declare_dram_parameter("cs4", [128, S], FP16, isOutput=False)
    sn4 = nc.declare_dram_parameter("sn4", [128, S], FP16, isOutput=False)
    sels = nc.declare_dram_parameter("sels", [24, 516], FP16, isOutput=False)
    selk = nc.declare_dram_parameter("selk", [32, NIF * 128], FP16,
                                     isOutput=False)
    masku = nc.declare_dram_parameter("masku", [128, max(n_pat, 1) * 256],
                                      FP16, isOutput=False)
    y = nc.declare_dram_parameter("y", [2 * TD, D], F32, isOutput=True)

    cc_in = [nc.dram_tensor(f"cc{i}_in", [NCORES, QF + QH, TD], FP16)
             for i in range(2)]
    cc_out = [nc.dram_tensor(f"cc{i}_out", [NCORES, QF + QH, TD], FP16)
              for i in range(2)]

    with tile.TileContext(nc) as tc:
        _emit(nc, tc, locals(), cls, n_pat)
    nc.finalize()
    return nc


def _emit(nc, tc, t, cls, n_pat):
    import contextlib
    ctx = contextlib.ExitStack()
    with ctx:
        persist = ctx.enter_context(tc.tile_pool(name="persist", bufs=1))

        # ---- persistent weights ----
        wqT_sb = persist.tile([128, NIF, QF], FP16)
        nc.scalar.dma_start(out=wqT_sb,
                            in_=t["wqT"].rearrange("(n p) f -> p n f", p=128))
        wkvT_sb = persist.tile([128, NIF, 128], FP16)
        nc.scalar.dma_start(out=wkvT_sb,
                            in_=t["wkvT"].rearrange("(n p) f -> p n f", p=128))
        aqk_sb = persist.tile([128, NIF, 128], FP16)
        nc.gpsimd.dma_start(out=aqk_sb,
                            in_=t["aqk"].rearrange("(n p) f -> p n f", p=128))
        avr_sb = persist.tile([128, NIF, 88], FP16)
        nc.gpsimd.dma_start(out=avr_sb,
                            in_=t["avr"].rearrange("(n p) f -> p n f", p=128))
        cs4_sb = persist.tile([128, S], FP16)
        nc.gpsimd.dma_start(out=cs4_sb, in_=t["cs4"][:])
        sn4_sb = persist.tile([128, S], FP16)
        nc.gpsimd.dma_start(out=sn4_sb, in_=t["sn4"][:])
        aob_sb = persist.tile([128, NIF, 72], FP16)
        nc.gpsimd.dma_start(out=aob_sb,
                            in_=t["aob"].rearrange("(n p) f -> p n f", p=128))
        bq_sb = persist.tile([64, QF], FP16)
        nc.gpsimd.dma_start(out=bq_sb, in_=t["bq"][:])
        bkv_sb = persist.tile([128, 64], FP16)
        nc.gpsimd.dma_start(out=bkv_sb, in_=t["bkv"][:])
        bv_sb = persist.tile([64, 64], FP16)
        nc.gpsimd.dma_start(out=bv_sb, in_=t["bv"][:])
        bo_sb = persist.tile([64, D], FP16)
        nc.gpsimd.dma_start(out=bo_sb, in_=t["bo"][:])
        cs4_sb = persist.tile([128, S], FP16)
        nc.gpsimd.dma_start(out=cs4_sb, in_=t["cs4"][:])
        sn4_sb = persist.tile([128, S], FP16)
        nc.gpsimd.dma_start(out=sn4_sb, in_=t["sn4"][:])
        # sels columns: [0:128 sel_qk | 128:192 sel_v | 192:320 sel3_qk(3r) |
        #               320:384 pad | 384:387 ones_sel.T? -> see host packing]
        sels_sb = persist.tile([24, 516], FP16)
        nc.gpsimd.dma_start(out=sels_sb, in_=t["sels"][:])
        selk_sb = persist.tile([32, NIF, 128], FP16)
        nc.gpsimd.dma_start(
            out=selk_sb, in_=t["selk"].rearrange("h (n f) -> h n f", f=128))
        masku_sb = persist.tile([128, max(n_pat, 1), 256], FP16)
        nc.gpsimd.dma_start(
            out=masku_sb,
            in_=t["masku"].rearrange("p (u f) -> p u f", f=256))

        sel_qk = sels_sb[:, 0:128]
        sel_v = sels_sb[:, 128:192]
        ones_blk = sels_sb[:, 192:216]   # [24, 24] block-diag ones
        sel_o = sels_sb[0:8, 216:280]
        ones8x8 = sels_sb[0:8, 280:288]  # [8, 8] all ones

        ident_h = persist.tile([64, 64], FP16)
        make_identity(nc, ident_h)
        ebias = persist.tile([128, 1], F32)
        nc.vector.memset(ebias, EXP_BIAS)

        qh_sb = persist.tile([64, QH, S], FP16)
        kh_sb = persist.tile([64, S], FP16)
        vT_sb = persist.tile([64, S], FP16)
        vtok = persist.tile([128, NKT, 65], FP16)
        nc.vector.memset(vtok[:, :, 64:65], 1.0)
        woT_sb = persist.tile([128, NIF, D], FP16)  # DMAs issued after A

        # ================= Phase A =================
        with nc.named_scope("phaseA"), \
                tc.tile_pool(name="pA", bufs=1) as pA, \
                tc.tile_pool(name="psA", bufs=1, space="PSUM") as ps:
            xv = t["xT"].rearrange("(n p) s -> p n s", p=128)
            for tb in range(4):
                tsl = slice(tb * 512, (tb + 1) * 512)
                # quarter tiles so the first k-chunks unblock the PE early
                xqs = []
                for qtr in range(4):
                    xqt = pA.tile([128, 4, 512], FP16, name="xq%d" % qtr,
                                  tag="xq%d" % qtr, bufs=2)
                    nc.sync.dma_start(
                        out=xqt, in_=xv[:, 4 * qtr:4 * qtr + 4, tsl])
                    xqs.append(xqt)

                def xq_(k):
                    return xqs[k // 4][:, k % 4, :]

                la = ps.tile([128, 512], F32, name="la", tag="b_la")
                lv = ps.tile([88, 512], F32, name="lv", tag="b_lv")
                q0 = ps.tile([128, 512], F32, name="q0", tag="b_q0")
                q1 = ps.tile([128, 512], F32, name="q1", tag="b_q1")
                kv = ps.tile([128, 512], F32, name="kv", tag="b_kv")
                lsc = ps.tile([128, 512], F32, name="lsc", tag="b_lsc")
                rwx = ps.tile([128, 512], F32, name="rwx", tag="b_rwx")
                vps = ps.tile([128, 2, 64], FP16, name="vps", tag="b_vps")

                for k in range(NIF):
                    st, sp = k == 0, k == NIF - 1
                    nc.tensor.matmul(la, aqk_sb[:, k, :], xq_(k),
                                     start=st, stop=sp)
                for k in range(NIF):
                    st, sp = k == 0, k == NIF - 1
                    nc.tensor.matmul(lv, avr_sb[:, k, :], xq_(k),
                                     start=st, stop=sp)
                # router softmax (fp32, no max-subtract; logits lv[64:88])
                ex_h = pA.tile([24, 512], FP16, name="ex_h", tag="ex_h",
                               bufs=2)
                nc.scalar.activation(ex_h, lv[64:88, :], AF.Exp)

                for k in range(NIF):
                    nc.tensor.matmul(q0, wqT_sb[:, k, 0:128], xq_(k),
                                     start=(k == 0), stop=False)
                # per-group softmax sums at 24 partitions -> lsc rows 0:24
                nc.tensor.matmul(lsc[0:24, :], ones_blk, ex_h,
                                 start=True, stop=True)
                rec24 = pA.tile([24, 512], FP16, name="rec24", tag="rec24",
                                bufs=2)
                with nc.allow_low_precision(reason="router softmax recip"):
                    nc.vector.reciprocal(rec24, lsc[0:24, :])
                rw_n = pA.tile([24, 512], FP16, name="rw_n", tag="rw_n",
                               bufs=2)
                nc.vector.tensor_tensor(rw_n, ex_h, rec24, AluOpType.mult)

                for k in range(NIF):
                    nc.tensor.matmul(q1, wqT_sb[:, k, 128:256], xq_(k),
                                     start=(k == 0), stop=False)
                nc.tensor.matmul(rwx, sel_qk, rw_n, start=True, stop=True)
                la_sb = pA.tile([128, 512], FP16, name="la_sb", tag="la_sb",
                                bufs=2)
                nc.vector.tensor_copy(la_sb, la)
                hpqk = pA.tile([128, 512], FP16, name="hpqk", tag="hpqk",
                               bufs=2)
                nc.vector.tensor_tensor(hpqk, la_sb, rwx, AluOpType.mult)

                for k in range(NIF):
                    nc.tensor.matmul(kv, wkvT_sb[:, k, :], xq_(k),
                                     start=(k == 0), stop=False)
                # v-lora combine (reuses the lsc bank after rec24's read)
                nc.tensor.matmul(lsc[0:64, :], sel_v, rw_n,
                                 start=True, stop=True)
                lv_sb = pA.tile([64, 512], FP16, name="lv_sb", tag="lv_sb",
                                bufs=2)
                nc.vector.tensor_copy(lv_sb, lv[0:64, :])
                hpv = pA.tile([64, 512], FP16, name="hpv", tag="hpv", bufs=2)
                nc.vector.tensor_tensor(hpv, lv_sb, lsc[0:64, :],
                                        AluOpType.mult)

                # LoRA-B accumulations
                nc.tensor.matmul(q0, bq_sb[:, 0:128], hpqk[0:64, :],
                                 start=False, stop=True)
                nc.tensor.matmul(q1, bq_sb[:, 128:256], hpqk[0:64, :],
                                 start=False, stop=True)
                nc.tensor.matmul(kv[0:64, :], bkv_sb[64:128, :],
                                 hpqk[64:128, :], start=False, stop=True)
                nc.tensor.matmul(kv[64:128, :], bv_sb, hpv,
                                 start=False, stop=True)

                # ---- RoPE (fp16) ----
                q0c = pA.tile([128, 512], FP16, name="q0c", tag="q0c", bufs=2)
                nc.vector.tensor_copy(q0c, q0)
                q1c = pA.tile([128, 512], FP16, name="q1c", tag="q1c", bufs=2)
                nc.scalar.activation(q1c, q1, AF.Copy)
                csl = cs4_sb[:, tsl]
                snl = sn4_sb[:, tsl]
                t1 = pA.tile([128, 512], FP16, name="t1", tag="t1", bufs=2)
                t2 = pA.tile([128, 512], FP16, name="t2", tag="t2", bufs=2)
                qre = pA.tile([128, 512], FP16, name="qre", tag="qre", bufs=2)
                qro = pA.tile([128, 512], FP16, name="qro", tag="qro", bufs=2)
                nc.vector.tensor_tensor(t1, q0c, csl, AluOpType.mult)
                nc.vector.tensor_tensor(t2, q1c, snl, AluOpType.mult)
                nc.vector.tensor_tensor(qre, t1, t2, AluOpType.subtract)
                nc.vector.tensor_tensor(t1, q0c, snl, AluOpType.mult)
                nc.vector.tensor_tensor(t2, q1c, csl, AluOpType.mult)
                nc.vector.tensor_tensor(qro, t1, t2, AluOpType.add)

                kc2 = pA.tile([32, 2, 512], FP16, name="kc2", tag="kc2",
                              bufs=2)
                nc.vector.tensor_copy(kc2[:, 0, :], kv[0:32, :])
                nc.vector.tensor_copy(kc2[:, 1, :], kv[32:64, :])
                nc.scalar.activation(vT_sb[:, tsl], kv[64:128, :], AF.Copy)
                csl32 = cs4_sb[0:32, tsl]
                snl32 = sn4_sb[0:32, tsl]
                tk1 = pA.tile([32, 512], FP16, name="tk1", tag="tk1", bufs=2)
                tk2 = pA.tile([32, 512], FP16, name="tk2", tag="tk2", bufs=2)
                kho = pA.tile([32, 512], FP16, name="kho", tag="kho", bufs=2)
                nc.vector.tensor_tensor(tk1, kc2[:, 0, :], csl32,
                                        AluOpType.mult)
                nc.vector.tensor_tensor(tk2, kc2[:, 1, :], snl32,
                                        AluOpType.mult)
                nc.vector.tensor_tensor(kh_sb[0:32, tsl], tk1, tk2,
                                        AluOpType.subtract)
                nc.vector.tensor_tensor(tk1, kc2[:, 0, :], snl32,
                                        AluOpType.mult)
                nc.vector.tensor_tensor(tk2, kc2[:, 1, :], csl32,
                                        AluOpType.mult)
                nc.vector.tensor_tensor(kho, tk1, tk2, AluOpType.add)
                nc.vector.tensor_copy(kh_sb[32:64, tsl], kho)

                for h in range(QH):
                    nc.sync.dma_start(
                        out=qh_sb[0:32, h, tsl],
                        in_=qre[32 * h:32 * h + 32, :])
                    nc.sync.dma_start(
                        out=qh_sb[32:64, h, tsl],
                        in_=qro[32 * h:32 * h + 32, :])

                # token-major v
                for j in range(4):
                    kt = 4 * tb + j
                    nc.tensor.transpose(
                        vps[:, j % 2, :], vT_sb[:, 128 * kt:128 * kt + 128],
                        ident_h)
                    nc.vector.tensor_copy(vtok[:, kt, 0:64], vps[:, j % 2, :])

        # prefetch wo during attention
        for j, eng in enumerate((nc.sync, nc.scalar, nc.gpsimd, nc.sync)):
            eng.dma_start(
                out=woT_sb[:, :, 512 * j:512 * j + 512],
                in_=t["woT"].rearrange("(n p) f -> p n f", p=128)
                [:, :, 512 * j:512 * j + 512])

        if KDBG == "qkv":
            with tc.tile_pool(name="pX", bufs=1) as pX:
                for r, src in enumerate((qh_sb[:, 0, :], kh_sb, vT_sb,
                                         qh_sb[:, 1, :])):
                    ytd = pX.tile([64, S], F32, name="ytd", tag="ytd", bufs=2)
                    nc.vector.tensor_copy(ytd, src)
                    nc.sync.dma_start(out=t["y"][64 * r:64 * r + 64, :],
                                      in_=ytd)
            return

        # ================= Phase C =================
        sc_i = 0
        with nc.named_scope("phaseC"), \
                tc.tile_pool(name="pC", bufs=1) as pC, \
                tc.tile_pool(name="psC", bufs=1, space="PSUM") as ps:
            for qb in range(NQB):
                outp = ps.tile([65, 2, 2, 2, 256], F32, name="outp",
                               tag="outp")
                first = {}
                last = {}
                for hf in range(2):
                    act = [kt for kt in range(NKT)
                           if cls[kt, qb, hf] != M_SKIP]
                    first[hf], last[hf] = act[0], act[-1]
                for kt in range(4 * qb + 4):
                    ksl = slice(128 * kt, 128 * kt + 128)
                    for hf in range(2):
                        c = cls[kt, qb, hf]
                        if c == M_SKIP:
                            continue
                        qsl = slice(512 * qb + 256 * hf,
                                    512 * qb + 256 * hf + 256)
                        sc = ps.tile([128, 2, 2, 256], F32, name="sc",
                                     tag="b_sc%d" % (sc_i % 2))
                        sc_i += 1
                        for p in range(2):
                            nc.tensor.matmul(
                                sc[:, p, :, :], kh_sb[:, ksl],
                                qh_sb[:, 2 * p:2 * p + 2, qsl],
                                start=True, stop=True)
                        if c >= 0:
                            mt = masku_sb[:, c, :]
                            nc.vector.tensor_tensor(
                                sc, sc,
                                mt.unsqueeze(1).unsqueeze(1)
                                .broadcast_to([128, 2, 2, 256]),
                                AluOpType.add)
                        pr = pC.tile([128, 2, 2, 256], FP16, name="pr",
                                     tag="pr", bufs=3)
                        nc.scalar.activation(pr, sc, AF.Exp, bias=ebias)
                        for p in range(2):
                            nc.tensor.matmul(
                                outp[:, p, hf, :, :], vtok[:, kt, :],
                                pr[:, p, :, :],
                                start=(kt == first[hf]),
                                stop=(kt == last[hf]))
                on65 = pC.tile([65, QH, 512], FP16, name="on65", tag="on65",
                               bufs=2)
                for hf in range(2):
                    nc.vector.tensor_copy(
                        on65[:, :, 256 * hf:256 * hf + 256]
                        .rearrange("P (a i) t -> P a i t", i=2),
                        outp[:, :, hf, :, :])
                if KDBG == "att":
                    if qb == 0:
                        ytd = pC.tile([65, QH * 512], F32, name="ytd")
                        nc.vector.tensor_copy(
                            ytd, on65.rearrange("P h t -> P (h t)"))
                        nc.sync.dma_start(out=t["y"][0:65, :], in_=ytd)
                        ytd2 = pC.tile([128, NKT * 65], F32, name="ytd2")
                        nc.vector.tensor_copy(
                            ytd2, vtok.rearrange("p k c -> p (k c)"))
                        nc.sync.dma_start(
                            out=t["y"][128:256, 0:NKT * 65], in_=ytd2)
                    continue
                cci = t["cc_in"][qb // 2]
                for j in range(4):
                    d = 4 * (qb % 2) + j
                    tj = slice(128 * j, 128 * j + 128)
                    nc.gpsimd.dma_start(
                        out=cci[d, 0:QF, :].rearrange("(h p) t -> p h t",
                                                      p=64),
                        in_=on65[0:64, :, tj])
                    for h in range(QH):
                        nc.gpsimd.dma_start(
                            out=cci[d, QF + h:QF + h + 1, :],
                            in_=on65[64:65, h, tj])
                if qb % 2 == 1 and KDBG != "att":
                    i = qb // 2
                    nc.gpsimd.collective_compute(
                        "AllToAll", AluOpType.bypass,
                        ins=[t["cc_in"][i][:]],
                        outs=[t["cc_out"][i][:]],
                        replica_groups=[list(range(NCORES))],
                    )

        if KDBG == "att":
            return
        # ================= Phase D =================
        with nc.named_scope("phaseD"), \
                tc.tile_pool(name="pD", bufs=1) as pD, \
                tc.tile_pool(name="psD", bufs=1, space="PSUM") as ps:
            for i in range(2):
                cco = t["cc_out"][i]
                g = pD.tile([128, NIF, TD], FP16, name="g", tag="g%d" % i)
                for j in range(2):
                    nc.sync.dma_start(
                        out=g.rearrange("p (c j) t -> p c j t", j=2)
                        [:, :, j, :],
                        in_=cco[:, 128 * j:128 * j + 128, :]
                        .rearrange("c p t -> p c t"))
                den = pD.tile([32, TD], FP16, name="den", tag="den", bufs=2)
                for cb in range(NCORES):
                    nc.sync.dma_start(
                        out=den[QH * cb:QH * cb + QH, :],
                        in_=cco[cb, QF:QF + QH, :])
                if KDBG == "gd":
                    gtd = pD.tile([128, NIF * TD], F32, name="gtd",
                                  tag="gtd", bufs=2)
                    nc.vector.tensor_copy(
                        gtd, g.rearrange("p k t -> p (k t)"))
                    if i == 0:
                        nc.sync.dma_start(out=t["y"][0:128, :], in_=gtd)
                    dtd = pD.tile([32, TD], F32, name="dtd", tag="dtd",
                                  bufs=2)
                    nc.vector.tensor_copy(dtd, den)
                    nc.sync.dma_start(
                        out=t["y"][128 + 32 * i:160 + 32 * i, 0:TD], in_=dtd)
                    continue
                rec = pD.tile([32, TD], FP16, name="rec", tag="rec", bufs=2)
                with nc.allow_low_precision(reason="attn denom recip"):
                    nc.vector.reciprocal(rec, den)
                # software-pipelined: normalize chunk k, then immediately its
                # Y and o-lora-A matmuls (Y streams while DVE normalizes k+1)
                Y = ps.tile([128, 4, 512], F32, name="Y", tag="b_Y")
                ho = ps.tile([128, TD], F32, name="ho", tag="b_ho")
                for k in range(NIF):
                    rb = ps.tile([128, TD], F32, name="rb",
                                 tag="b_rb%d" % (k % 2))
                    nc.tensor.matmul(rb, selk_sb[:, k, :], rec,
                                     start=True, stop=True)
                    nc.vector.tensor_tensor(g[:, k, :], g[:, k, :], rb,
                                            AluOpType.mult)
                    for ob in range(4):
                        nc.tensor.matmul(
                            Y[:, ob, :], g[:, k, :],
                            woT_sb[:, k, 512 * ob:512 * ob + 512],
                            start=(k == 0), stop=False)
                    nc.tensor.matmul(ho[0:72, :], aob_sb[:, k, :], g[:, k, :],
                                     start=(k == 0), stop=(k == NIF - 1))
                ex_o = pD.tile([8, TD], FP16, name="ex_o", tag="ex_o", bufs=2)
                nc.scalar.activation(ex_o, ho[64:72, :], AF.Exp)
                sm_o = ps.tile([128, TD], F32, name="sm_o", tag="b_rb0")
                nc.tensor.matmul(sm_o[0:8, :], ones8x8, ex_o,
                                 start=True, stop=True)
                rec_o = pD.tile([8, TD], FP16, name="rec_o", tag="rec_o",
                                bufs=2)
                with nc.allow_low_precision(reason="o router recip"):
                    nc.vector.reciprocal(rec_o, sm_o[0:8, :])
                rw_o = pD.tile([8, TD], FP16, name="rw_o", tag="rw_o", bufs=2)
                nc.vector.tensor_tensor(rw_o, ex_o, rec_o, AluOpType.mult)
                rt = ps.tile([64, TD], F32, name="rt", tag="b_rt")
                nc.tensor.matmul(rt, sel_o, rw_o, start=True, stop=True)
                ho_sb = pD.tile([64, TD], FP16, name="ho_sb", tag="ho_sb",
                                bufs=2)
                nc.vector.tensor_copy(ho_sb, ho[0:64, :])
                hpo = pD.tile([64, TD], FP16, name="hpo", tag="hpo", bufs=2)
                nc.vector.tensor_tensor(hpo, ho_sb, rt, AluOpType.mult)

                for ob in range(4):
                    nc.tensor.matmul(Y[:, ob, :], hpo,
                                     bo_sb[:, 512 * ob:512 * ob + 512],
                                     start=False, stop=True)
                yt = pD.tile([128, 4, 512], F32, name="yt", tag="yt", bufs=2)
                nc.vector.tensor_copy(yt[:, 0, :], Y[:, 0, :])
                nc.scalar.activation(yt[:, 1, :], Y[:, 1, :], AF.Copy)
                nc.vector.tensor_copy(yt[:, 2, :], Y[:, 2, :])
                nc.scalar.activation(yt[:, 3, :], Y[:, 3, :], AF.Copy)
                nc.sync.dma_start(
                    out=t["y"][TD * i:TD * i + TD, 0:1024],
                    in_=yt[:, 0:2, :].rearrange("p a f -> p (a f)"))
                nc.scalar.dma_start(
                    out=t["y"][TD * i:TD * i + TD, 1024:2048],
                    in_=yt[:, 2:4, :].rearrange("p a f -> p (a f)"))


# ======================= host side =======================

_CACHE = {}


def _prep_inputs(x, mask, freqs_cos, freqs_sin, wq, wk, wv, wo,
                 lq_router, lq_A, lq_B, lk_router, lk_A, lk_B,
                 lv_router, lv_A, lv_B, lo_router, lo_A, lo_B):
    scale = 1.0 / np.sqrt(HD)
    x = _f32(np.asarray(x)).reshape(S, D)
    maskf = _f32(np.asarray(mask)).reshape(S, S)
    maskT = np.maximum(maskf, MASK_NEG).T.copy()
    cls, patterns = classify_mask(maskT)
    n_pat = len(patterns)
    if n_pat:
        masku = np.stack(patterns, axis=1).reshape(128, n_pat * 256)
    else:
        masku = np.zeros((128, 256), dtype=np.float32)

    cos = _f32(freqs_cos)  # [S, 32]
    sin = _f32(freqs_sin)
    cs4 = _fp16(np.tile(cos.T, (4, 1)))
    sn4 = _fp16(np.tile(sin.T, (4, 1)))

    # selector pack [24, 516] (cols: sel_qk 0:128 | sel_v 128:192 |
    #   ones_blk 192:216 | sel_o 216:280 | ones8x8 280:288)
    sels = np.zeros((24, 516), dtype=np.float32)
    for e in range(E):
        for r in range(R):
            sels[e, r * 8 + e] = 1.0                  # sel_qk (q)
            sels[8 + e, 64 + r * 8 + e] = 1.0         # sel_qk (k)
            sels[16 + e, 128 + r * 8 + e] = 1.0       # sel_v
            sels[e, 216 + r * 8 + e] = 1.0            # sel_o
    for j in range(24):
        sels[j, 192 + (j // 8) * 8:192 + (j // 8) * 8 + 8] = 1.0  # ones_blk
    sels[0:8, 280:288] = 1.0                          # ones8x8

    selkm = np.zeros((32, NIF * 128), dtype=np.float32)
    for k in range(NIF):
        for p in range(128):
            selkm[2 * k + p // 64, 128 * k + p] = 1.0

    ao_p = np.concatenate([_a_pack(_f32(lo_A)), _f32(lo_router).T], axis=1)
    shared = dict(xT=_fp16(x.T), cs4=cs4, sn4=sn4, woT=_fp16(_f32(wo).T),
                  masku=_fp16(masku), sels=_fp16(sels), selk=_fp16(selkm),
                  aob=_fp16(ao_p), bo=_fp16(_b_flat(_f32(lo_B), SCALING)))

    aq_p = _a_pack(_f32(lq_A))
    ak_p = _a_pack(_f32(lk_A))
    av_p = _a_pack(_f32(lv_A))
    aqk_p = _fp16(np.concatenate([aq_p, ak_p], axis=1))
    avr_p = _fp16(np.concatenate(
        [av_p, _f32(lq_router).T, _f32(lk_router).T, _f32(lv_router).T],
        axis=1))

    wqf, wkf, wvf = _f32(wq), _f32(wk), _f32(wv)
    lqB, lkB, lvB = _f32(lq_B), _f32(lk_B), _f32(lv_B)

    in_maps = []
    for c in range(NCORES):
        wq_c = wqf[c * QF:(c + 1) * QF][IDX_Q] * scale
        wk_c = wkf[c * HD:(c + 1) * HD][IDX_K]
        wv_c = wvf[c * HD:(c + 1) * HD]
        wkv_c = np.concatenate([wk_c, wv_c], axis=0)
        bq_c = _b_flat(lqB[:, c * QF:(c + 1) * QF, :][:, IDX_Q, :],
                       SCALING * scale)
        bk_c = _b_flat(lkB[:, c * HD:(c + 1) * HD, :][:, IDX_K, :], SCALING)
        bkv_c = np.zeros((128, 64), dtype=np.float32)
        bkv_c[64:128] = bk_c
        bv_c = _b_flat(lvB[:, c * HD:(c + 1) * HD, :], SCALING)
        m = dict(shared)
        m.update(wqT=_fp16(wq_c.T), wkvT=_fp16(wkv_c.T),
                 aqk=aqk_p, avr=avr_p,
                 bq=_fp16(bq_c), bkv=_fp16(bkv_c), bv=_fp16(bv_c))
        in_maps.append(m)
    return in_maps, cls, n_pat


def get_graph(cls, n_pat):
    key = (cls.tobytes(), n_pat, KDBG)
    if key not in _CACHE:
        _CACHE[key] = build(cls, n_pat)
    return _CACHE[key]


def kernel(x, start_pos, mask, freqs_cos, freqs_sin, wq, wk, wv, wo,
           lq_router, lq_A, lq_B, lk_router, lk_A, lk_B,
           lv_router, lv_A, lv_B, lo_router, lo_A, lo_B,
           _trace=False):
    from concourse.bass_utils import run_bass_kernel_spmd
    in_maps, cls, n_pat = _prep_inputs(
        x, mask, freqs_cos, freqs_sin, wq, wk, wv, wo,
        lq_router, lq_A, lq_B, lk_router, lk_A, lk_B,
        lv_router, lv_A, lv_B, lo_router, lo_A, lo_B)
    nc = get_graph(cls, n_pat)
    res = run_bass_kernel_spmd(nc, in_maps, list(range(NCORES)), trace=_trace)
    out = np.empty((S, D), dtype=np.float32)
    for c in range(NCORES):
        yc = res.results[c]["y"]
        out[TD * c:TD * c + TD] = yc[0:TD]
        out[S // 2 + TD * c:S // 2 + TD * c + TD] = yc[TD:2 * TD]
    out = out.reshape(B, S, H * HD)
    if _trace:
        return out, res
    return out
